# revision 54
# baseline (speedup 1.0000x reference)
"""Trainium2 Bass kernel for the DifferentiableDroneController problem.

Strategy:
  - Pure data parallelism across 8 NeuronCores (batch split).
  - Host-side SoA repack: every per-row channel becomes a contiguous
    [128, F] plane on the device, so all device ops are dense
    elementwise plane ops (no strided access anywhere).
  - All transcendentals on the Scalar engine using only TWO activation
    table sets:
      * trig_and_small: Sin (cos via bias=pi/2), Arctan (arcsin via
        arctan(x*rsqrt(1-x^2)))
      * natural_log_exp_and_others: 1/x = exp(-ln x),
        rsqrt(x) = exp(-0.5 ln x), sqrt(x) = exp(0.5 ln x)
  - Vector engine uses fused ops: scalar_tensor_tensor (a*c + b in one
    instruction) and dual-op tensor_scalar (clip in one instruction).
  - Algebraic simplifications: I_VEC cancels out of omega_dot entirely,
    gravity is folded into constants, the acc-norm clamp is
    min(1, 10*rsqrt(s)).
"""

import math
from contextlib import ExitStack

import numpy as np

import concourse.bacc as bacc
import concourse.bass as bass
import concourse.mybir as mybir
from concourse import tile
from concourse.bass_utils import run_bass_kernel_spmd

AF = mybir.ActivationFunctionType
OP = mybir.AluOpType
F32 = mybir.dt.float32

P = 128
N_CORES = 8
B_TOTAL = 2_000_000
DT = 0.01
DT2 = 0.005
DT6 = DT / 6.0
G = 9.81
PI = math.pi
HPI = math.pi / 2.0
LN10 = math.log(10.0)
LN005 = math.log(0.05)

# channel indices in the packed input [34, P, F]
PX, PY, PZ = 0, 1, 2
VX, VY, VZ = 3, 4, 5
AR, AP_, AY = 6, 7, 8
OX, OY, OZ = 9, 10, 11
TPX, TPY, TPZ = 12, 13, 14
TYAW = 15
WDX, WDY, WDZ = 16, 17, 18
PVX, PVY, PVZ = 19, 20, 21
PRX, PRY, PRZ = 22, 23, 24
IPX, IPY, IPZ = 25, 26, 27
IVX, IVY, IVZ = 28, 29, 30
IRX, IRY, IRZ = 31, 32, 33

N_IN = 34
N_OUT = 12


class Arena:
    """Manual plane allocator over a single [P, NA*W] SBUF tile."""

    def __init__(self, ap, W, n_slots, n_reserved):
        self.ap = ap
        self.W = W
        self.free_list = list(range(n_reserved, n_slots))
        self.peak = n_reserved
        self.n_slots = n_slots
        self.live = n_reserved

    def plane(self, slot):
        W = self.W
        return self.ap[:, slot * W : (slot + 1) * W]

    def alloc(self):
        assert self.free_list, "arena exhausted"
        s = self.free_list.pop(0)
        self.live += 1
        self.peak = max(self.peak, self.n_slots - len(self.free_list))
        return s

    def free(self, *slots):
        for s in slots:
            assert s not in self.free_list
            self.free_list.append(s)
            self.live -= 1


def build_nc(F, W, gains, use_gpsimd=True):
    """Build the Bass program. gains: dict with kp(3), kip(3), c1v(3),
    c2v(3), c3v(3), katt(3), c1r(3), c2r(3), c3r(3) as python floats."""
    assert F % W == 0
    n_tiles = F // W

    kp = gains["kp"]
    kip = gains["kip"]
    c1v = gains["c1v"]
    c2v = gains["c2v"]
    c3v = gains["c3v"]
    katt = gains["katt"]
    c1r = gains["c1r"]
    c2r = gains["c2r"]
    c3r = gains["c3r"]

    NA = 66  # arena slots (measured peak liveness is 63)

    nc = bacc.Bacc()

    # The act-table-load inserter picks the FIRST table set containing a
    # function. By default that maps Ln -> natural_log (no exp) and
    # Exp -> exp_and_others (no ln), so every ln/exp pair costs two
    # ~1.7us table loads. Remove those functions from the suboptimal
    # sets in the cached table dict so both resolve to
    # natural_log_exp_and_others, and arctan to trig_and_small (shared
    # with sin). Indices of the remaining sets are unchanged, so the
    # emitted act_func_set_ids stay valid for walrus.
    from concourse.hw_specs import get_activation_tables

    tabs = get_activation_tables(nc.m.arch)
    tabs["sigmoid_and_others"].discard(AF.Arctan)

    # register const APs for the activation biases we use
    for cval in (HPI, G, DT2 * G, DT * G):
        cten = nc.alloc_sbuf_tensor(f"constu-f32-{cval}", [P, 1], F32)
        nc.gpsimd.memset(cten.ap(), cval)
        nc.const_aps.aps[(F32, cval)] = cten.ap()
    nc.all_engine_barrier()

    xin = nc.declare_dram_parameter("xin", [N_IN, P, F], F32, isOutput=False)
    yout = nc.declare_dram_parameter("yout", [N_OUT, P, F], F32, isOutput=True)

    with tile.TileContext(nc) as tc, ExitStack() as ctx:
        arena_pool = ctx.enter_context(tc.tile_pool(name="arena", bufs=1))
        out_pool = ctx.enter_context(tc.tile_pool(name="out", bufs=1))
        arena_tile = arena_pool.tile([P, NA * W], F32)

        V = nc.vector
        A = nc.scalar
        Gp = nc.gpsimd

        for t in range(n_tiles):
            ar = Arena(arena_tile[:], W, NA, N_IN)
            IN = [ar.plane(c) for c in range(N_IN)]

            # ---- load input planes (one DMA per channel keeps the
            # per-consumer sync-wait count within ISA limits) ----
            for c in range(N_IN):
                nc.sync.dma_start(
                    out=arena_tile[:, c * W : (c + 1) * W],
                    in_=xin[c, :, t * W : (t + 1) * W],
                )

            OUT = out_pool.tile([P, N_OUT * W], F32)

            def pl(s):
                return ar.plane(s)

            def new():
                return ar.alloc()

            # engine helpers
            def tt(op, a, b, out=None, eng=V):
                o = out if out is not None else new()
                eng.tensor_tensor(pl(o), pl(a), pl(b), op)
                return o

            def ts(a, s1, op0, s2=None, op1=None, out=None, eng=V):
                o = out if out is not None else new()
                if s2 is None:
                    eng.tensor_scalar(pl(o), pl(a), s1, None, op0)
                else:
                    eng.tensor_scalar(pl(o), pl(a), s1, s2, op0, op1)
                return o

            def stt(a, s, b, op0, op1, out=None, eng=V):
                o = out if out is not None else new()
                eng.scalar_tensor_tensor(pl(o), pl(a), s, pl(b), op0, op1)
                return o

            def act(a, func, bias=0.0, scale=1.0, out=None):
                o = out if out is not None else new()
                A.activation(pl(o), pl(a), func, bias=bias, scale=scale)
                return o

            def clip_ip(a, lo, hi):
                V.tensor_scalar(pl(a), pl(a), lo, hi, OP.max, OP.min)
                return a

            # ================= Phase T0: trig =================
            # target_yaw spans (-pi, pi): sin(x+pi/2) would leave the
            # HW sin domain [-pi, pi], so cos via 1 - 2*sin^2(y/2).
            sty = act(TYAW, AF.Sin)
            syh = act(TYAW, AF.Sin, scale=0.5)
            sy2q = act(syh, AF.Square)
            ar.free(syh)
            cty = ts(sy2q, -2.0, OP.mult, 1.0, OP.add)
            ar.free(sy2q)
            sr1 = act(AR, AF.Sin)
            cr1 = act(AR, AF.Sin, bias=HPI)
            sp1 = act(AP_, AF.Sin)
            cp1 = act(AP_, AF.Sin, bias=HPI)
            sy1 = act(AY, AF.Sin)
            cy1 = act(AY, AF.Sin, bias=HPI)

            # ================= controller part 1 (vector) =================
            dac = []
            for j, (pj, tpj, ipj, vj, ivj, pvj) in enumerate(
                [
                    (PX, TPX, IPX, VX, IVX, PVX),
                    (PY, TPY, IPY, VY, IVY, PVY),
                    (PZ, TPZ, IPZ, VZ, IVZ, PVZ),
                ]
            ):
                pe = tt(OP.subtract, tpj, pj)
                ip2 = stt(pe, DT, ipj, OP.mult, OP.add)
                clip_ip(ip2, -2.0, 2.0)
                q = act(pe, AF.Copy, scale=kp[j])
                ar.free(pe)
                dv = stt(ip2, kip[j], q, OP.mult, OP.add)
                ar.free(ip2, q)
                clip_ip(dv, -10.0, 10.0)
                ve = tt(OP.subtract, dv, vj)
                ar.free(dv)
                iv2 = stt(ve, DT, ivj, OP.mult, OP.add)
                clip_ip(iv2, -2.0, 2.0)
                q2 = act(ve, AF.Copy, scale=c1v[j])
                q3 = stt(iv2, c2v[j], q2, OP.mult, OP.add)
                ar.free(iv2, q2, ve)
                dacj = stt(pvj, -c3v[j], q3, OP.mult, OP.add)
                ar.free(q3)
                # controller-only input channels are dead now
                ar.free(tpj, ipj, ivj, pvj)
                dac.append(dacj)

            # v_rel for k1 (vector, independent)
            vr1 = [
                tt(OP.subtract, VX, WDX),
                tt(OP.subtract, VY, WDY),
                tt(OP.subtract, VZ, WDZ),
            ]
            # shifted wind-z for later stages (fold gravity)
            wdz_s2 = act(WDZ, AF.Identity, bias=DT2 * G)
            wdz_s4 = act(WDZ, AF.Identity, bias=DT * G)

            # ================= Phase L1: sqrt + reciprocal =========
            # acceleration-norm clamp factor = min(1, 10*rsqrt(s))
            sq0 = act(dac[0], AF.Square)
            sq1 = act(dac[1], AF.Square)
            sq2 = act(dac[2], AF.Square)
            sacc = tt(OP.add, sq0, sq1)
            tt(OP.add, sacc, sq2, out=sacc)
            ar.free(sq0, sq1, sq2)
            # sqrt(0.01*s) = 0.1*sqrt(s); recip -> 10/sqrt(s)
            racc = act(sacc, AF.Sqrt, scale=0.01)
            f10 = new()
            V.reciprocal(pl(f10), pl(racc))
            ar.free(sacc, racc)
            V.tensor_scalar(pl(f10), pl(f10), 1.0, None, OP.min)
            for j in range(3):
                tt(OP.mult, dac[j], f10, out=dac[j])
            ar.free(f10)
            # thrust vector: z gets +G (squares fold it via bias);
            # tn[:,2] is never used by the reference, so no explicit tvz.
            q0 = act(dac[0], AF.Square)
            q1 = act(dac[1], AF.Square)
            q2_ = act(dac[2], AF.Square, bias=G)
            s2 = tt(OP.add, q0, q1)
            tt(OP.add, s2, q2_, out=s2)
            ar.free(q0, q1, q2_)
            Tn = act(s2, AF.Sqrt)  # sqrt(s2) unclipped
            ar.free(s2)
            rsq2 = new()
            V.reciprocal(pl(rsq2), pl(Tn))
            T_ = ts(Tn, 0.1 * G, OP.max, 2.0 * G, OP.min)
            ar.free(Tn)
            tnx = tt(OP.mult, dac[0], rsq2)
            tny = tt(OP.mult, dac[1], rsq2)
            ar.free(dac[0], dac[1], dac[2], rsq2)
            # roll_arg
            ra = tt(OP.mult, tnx, sty)
            rb = tt(OP.mult, tny, cty)
            u = tt(OP.subtract, ra, rb)
            ar.free(ra, rb)
            # arcsin(u) = 2*arctan(u / (1 + sqrt(1-u^2))) keeps the
            # arctan argument inside the HW domain [-pi/2, pi/2].
            clip_ip(u, -0.999, 0.999)
            u2 = act(u, AF.Square)
            wu = act(u2, AF.Sqrt, bias=1.0, scale=-1.0)  # cos(des_roll)
            ar.free(u2)
            mru = new()
            V.reciprocal(pl(mru), pl(wu))  # 1/cos(des_roll)
            dnu = act(wu, AF.Identity, bias=1.0)
            ar.free(wu)
            rdu = new()
            V.reciprocal(pl(rdu), pl(dnu))
            ar.free(dnu)
            uarg = tt(OP.mult, u, rdu)
            ar.free(u, rdu)
            # pitch_arg
            pa = tt(OP.mult, tnx, cty)
            pb = tt(OP.mult, tny, sty)
            ar.free(tnx, tny, sty, cty)
            pc = tt(OP.add, pa, pb)
            v_ = tt(OP.mult, pc, mru)
            ar.free(pa, pb, pc, mru)
            clip_ip(v_, -0.999, 0.999)
            v2 = act(v_, AF.Square)
            wv = act(v2, AF.Sqrt, bias=1.0, scale=-1.0)
            ar.free(v2)
            dnv = act(wv, AF.Identity, bias=1.0)
            ar.free(wv)
            rdv = new()
            V.reciprocal(pl(rdv), pl(dnv))
            ar.free(dnv)
            varg = tt(OP.mult, v_, rdv)
            ar.free(v_, rdv)
            # k1 drag root and secant
            sv0 = act(vr1[0], AF.Square)
            sv1_ = act(vr1[1], AF.Square)
            sv2 = act(vr1[2], AF.Square)
            sv = tt(OP.add, sv0, sv1_)
            tt(OP.add, sv, sv2, out=sv)
            ar.free(sv0, sv1_, sv2)
            # 0.05*|v| = sqrt(0.0025*|v|^2)
            dro1 = act(sv, AF.Sqrt, scale=0.0025)
            ar.free(sv)
            sec1 = new()
            V.reciprocal(pl(sec1), pl(cp1))

            # ---------- dynamics helper (after trig + sec/dro ready) ----
            GE = Gp if use_gpsimd else V

            def att_dot(sr, cr, sp, sec, omx, omy, omz):
                m1 = tt(OP.mult, sr, omy, eng=GE)
                m2 = tt(OP.mult, cr, omz, eng=GE)
                m3 = tt(OP.add, m1, m2, eng=GE)
                ar.free(m1, m2)
                yd = tt(OP.mult, m3, sec, eng=GE)
                ar.free(m3)
                # roll_dot = wx + (sp*sec)*m3 = wx + sp*yd
                rda = tt(OP.mult, sp, yd, eng=GE)
                rd = tt(OP.add, rda, omx)
                ar.free(rda)
                pda = tt(OP.mult, cr, omy, eng=GE)
                pdb = tt(OP.mult, sr, omz, eng=GE)
                pd = tt(OP.subtract, pda, pdb, eng=GE)
                ar.free(pda, pdb)
                return rd, pd, yd

            def thrust_acc(sr, cr, sp, cp, sy, cy, dro, vr):
                t1 = tt(OP.mult, sp, cr, eng=GE)
                t2 = tt(OP.mult, cy, t1, eng=GE)
                t3 = tt(OP.mult, sy, sr, eng=GE)
                colx = tt(OP.add, t2, t3, eng=GE)
                ar.free(t2, t3)
                t4 = tt(OP.mult, sy, t1, eng=GE)
                t5 = tt(OP.mult, cy, sr, eng=GE)
                ar.free(t1)
                coly = tt(OP.subtract, t4, t5, eng=GE)
                ar.free(t4, t5)
                colz = tt(OP.mult, cp, cr, eng=GE)
                accs = []
                for colj, vrj in zip((colx, coly, colz), vr):
                    tg = tt(OP.mult, T_, colj, eng=GE)
                    ar.free(colj)
                    dr = tt(OP.mult, dro, vrj, eng=GE)
                    acc = tt(OP.subtract, tg, dr, eng=GE)
                    ar.free(tg, dr)
                    accs.append(acc)
                return accs

            def cross_xy(omx, omy, omz):
                # omega_dot_x = dtqx - cx, omega_dot_y = dtqy + cy;
                # the dtq part is hoisted into oxd2/oxd4 below, so only
                # the cross terms are computed per stage.
                cx = tt(OP.mult, omy, omz, eng=GE)
                cy_ = tt(OP.mult, omx, omz, eng=GE)
                return cx, cy_

            # k1 attitude dynamics (needs only state + sec1)
            rd1, pd1, yd1 = att_dot(sr1, cr1, sp1, sec1, OX, OY, OZ)
            # stage-2 attitude
            at2 = [
                stt(rd1, DT2, AR, OP.mult, OP.add),
                stt(pd1, DT2, AP_, OP.mult, OP.add),
                stt(yd1, DT2, AY, OP.mult, OP.add),
            ]

            # ================= Phase T2: trig =================
            droll = act(uarg, AF.Arctan)
            dpitch = act(varg, AF.Arctan)
            ar.free(uarg, varg)
            sr2 = act(at2[0], AF.Sin)
            cr2 = act(at2[0], AF.Sin, bias=HPI)
            sp2 = act(at2[1], AF.Sin)
            cp2 = act(at2[1], AF.Sin, bias=HPI)
            sy2 = act(at2[2], AF.Sin)
            cy2 = act(at2[2], AF.Sin, bias=HPI)
            ar.free(*at2)

            # ---- controller part 2 (vector) ----
            # droll/dpitch hold atan(tan(theta/2)); clip at 0.523/2 and
            # fold the *2 into the attitude-error subtraction.
            half_clip = float(np.float32(0.523) / np.float32(2.0))
            clip_ip(droll, -half_clip, half_clip)
            clip_ip(dpitch, -half_clip, half_clip)
            aer = stt(droll, 2.0, AR, OP.mult, OP.subtract)
            aep = stt(dpitch, 2.0, AP_, OP.mult, OP.subtract)
            ar.free(droll, dpitch)
            x_ = tt(OP.subtract, TYAW, AY)
            g1 = ts(x_, PI, OP.is_gt, 2.0 * PI, OP.mult)
            g2 = ts(x_, -PI, OP.is_lt, 2.0 * PI, OP.mult)
            x1 = stt(g1, -1.0, x_, OP.mult, OP.add)
            ar.free(g1, x_)
            aey = tt(OP.add, x1, g2)
            ar.free(x1, g2, TYAW)
            dtq = []
            for j, (aej, oj, irj, prj) in enumerate(
                [(aer, OX, IRX, PRX), (aep, OY, IRY, PRY), (aey, OZ, IRZ, PRZ)]
            ):
                re = stt(aej, katt[j], oj, OP.mult, OP.subtract)
                ar.free(aej)
                ir2 = stt(re, DT, irj, OP.mult, OP.add)
                clip_ip(ir2, -1.0, 1.0)
                q = act(re, AF.Copy, scale=c1r[j])
                q2 = stt(ir2, c2r[j], q, OP.mult, OP.add)
                ar.free(ir2, q, re)
                dtqj = stt(prj, -c3r[j], q2, OP.mult, OP.add)
                ar.free(q2, irj, prj)
                dtq.append(dtqj)

            # ---- finish k1 (vector) ----
            # hoisted omega + dt*dtq terms (shared by stages and final)
            oxd2 = stt(dtq[0], DT2, OX, OP.mult, OP.add)
            oyd2 = stt(dtq[1], DT2, OY, OP.mult, OP.add)
            acc1 = thrust_acc(sr1, cr1, sp1, cp1, sy1, cy1, dro1, vr1)
            ar.free(sr1, cr1, sp1, cp1, sy1, cy1, sec1, dro1, *vr1)
            c1x, c1y = cross_xy(OX, OY, OZ)
            # stage-2 velocity / omega
            ve2 = [
                stt(acc1[0], DT2, VX, OP.mult, OP.add),
                stt(acc1[1], DT2, VY, OP.mult, OP.add),
                stt(acc1[2], DT2, VZ, OP.mult, OP.add),
            ]
            om2 = [
                stt(c1x, -DT2, oxd2, OP.mult, OP.add),
                stt(c1y, DT2, oyd2, OP.mult, OP.add),
                stt(dtq[2], DT2, OZ, OP.mult, OP.add),
            ]

            # generic stage: given trig phase done for atI, compute
            # dynamics kI, accumulate, produce next stage state.
            ACC_p = [None, None, None]
            ACC_v = [None, None, None]
            ACC_a = [None, None, None]
            ACC_c = [None, None]  # weighted cross-term sums

            def ln_exp_phase(veI, atI_trig_cp, vrz_shift):
                """v_rel, drag root, secant for one stage."""
                vrI = [
                    tt(OP.subtract, veI[0], WDX, eng=GE),
                    tt(OP.subtract, veI[1], WDY, eng=GE),
                    tt(OP.subtract, veI[2], vrz_shift, eng=GE),
                ]
                s0 = act(vrI[0], AF.Square)
                s1 = act(vrI[1], AF.Square)
                s2_ = act(vrI[2], AF.Square)
                sv_ = tt(OP.add, s0, s1)
                tt(OP.add, sv_, s2_, out=sv_)
                ar.free(s0, s1, s2_)
                dro = act(sv_, AF.Sqrt, scale=0.0025)
                ar.free(sv_)
                sec = new()
                V.reciprocal(pl(sec), pl(atI_trig_cp))
                return vrI, dro, sec

            def accumulate(planes, weight, slot_list, base=None):
                """ACC update: ACC = base + weight*planes (init) or
                ACC += weight*planes."""
                for i_, p_ in enumerate(planes):
                    if slot_list[i_] is None:
                        # init: ACC = weight*p + base_i
                        slot_list[i_] = stt(
                            p_, weight, base[i_], OP.mult, OP.add
                        )
                    else:
                        if weight == 1.0:
                            tt(OP.add, slot_list[i_], p_, out=slot_list[i_])
                        else:
                            stt(
                                p_,
                                weight,
                                slot_list[i_],
                                OP.mult,
                                OP.add,
                                out=slot_list[i_],
                            )

            # ===== k2 =====
            vr2, dro2, sec2 = ln_exp_phase(ve2, cp2, wdz_s2)
            rd2, pd2, yd2 = att_dot(sr2, cr2, sp2, sec2, *om2)
            acc2 = thrust_acc(sr2, cr2, sp2, cp2, sy2, cy2, dro2, vr2)
            ar.free(sr2, cr2, sp2, cp2, sy2, cy2, sec2, dro2, *vr2)
            c2x, c2y = cross_xy(*om2)
            # stage-3 state
            at3 = [
                stt(rd2, DT2, AR, OP.mult, OP.add),
                stt(pd2, DT2, AP_, OP.mult, OP.add),
                stt(yd2, DT2, AY, OP.mult, OP.add),
            ]
            ve3 = [
                stt(acc2[0], DT2, VX, OP.mult, OP.add),
                stt(acc2[1], DT2, VY, OP.mult, OP.add),
                stt(acc2[2], DT2, VZ, OP.mult, OP.add),
            ]
            om3 = [
                stt(c2x, -DT2, oxd2, OP.mult, OP.add),
                stt(c2y, DT2, oyd2, OP.mult, OP.add),
                stt(dtq[2], DT2, OZ, OP.mult, OP.add),
            ]
            # ACC init with k1 + 2*k2
            accumulate(ve2, 2.0, ACC_p, base=[VX, VY, VZ])
            accumulate(acc2, 2.0, ACC_v, base=acc1)
            accumulate([rd2, pd2, yd2], 2.0, ACC_a, base=[rd1, pd1, yd1])
            accumulate([c2x, c2y], 2.0, ACC_c, base=[c1x, c1y])
            ar.free(*ve2, *om2, *acc1, *acc2, rd1, pd1, yd1, rd2, pd2, yd2)
            ar.free(c1x, c1y, c2x, c2y, oxd2, oyd2)

            # ================= Phase T4: trig =================
            sr3 = act(at3[0], AF.Sin)
            cr3 = act(at3[0], AF.Sin, bias=HPI)
            sp3 = act(at3[1], AF.Sin)
            cp3 = act(at3[1], AF.Sin, bias=HPI)
            sy3 = act(at3[2], AF.Sin)
            cy3 = act(at3[2], AF.Sin, bias=HPI)
            ar.free(*at3)

            # ===== k3 =====
            vr3, dro3, sec3 = ln_exp_phase(ve3, cp3, wdz_s2)
            rd3, pd3, yd3 = att_dot(sr3, cr3, sp3, sec3, *om3)
            acc3 = thrust_acc(sr3, cr3, sp3, cp3, sy3, cy3, dro3, vr3)
            ar.free(sr3, cr3, sp3, cp3, sy3, cy3, sec3, dro3, *vr3)
            c3x, c3y = cross_xy(*om3)
            oxd4 = stt(dtq[0], DT, OX, OP.mult, OP.add)
            oyd4 = stt(dtq[1], DT, OY, OP.mult, OP.add)
            # stage-4 state (full dt)
            at4 = [
                stt(rd3, DT, AR, OP.mult, OP.add),
                stt(pd3, DT, AP_, OP.mult, OP.add),
                stt(yd3, DT, AY, OP.mult, OP.add),
            ]
            ve4 = [
                stt(acc3[0], DT, VX, OP.mult, OP.add),
                stt(acc3[1], DT, VY, OP.mult, OP.add),
                stt(acc3[2], DT, VZ, OP.mult, OP.add),
            ]
            om4 = [
                stt(c3x, -DT, oxd4, OP.mult, OP.add),
                stt(c3y, DT, oyd4, OP.mult, OP.add),
                stt(dtq[2], DT, OZ, OP.mult, OP.add),
            ]
            accumulate(ve3, 2.0, ACC_p)
            accumulate(acc3, 2.0, ACC_v)
            accumulate([rd3, pd3, yd3], 2.0, ACC_a)
            accumulate([c3x, c3y], 2.0, ACC_c)
            ar.free(*ve3, *om3, *acc3, rd3, pd3, yd3, c3x, c3y)

            # ================= Phase T6: trig =================
            sr4 = act(at4[0], AF.Sin)
            cr4 = act(at4[0], AF.Sin, bias=HPI)
            sp4 = act(at4[1], AF.Sin)
            cp4 = act(at4[1], AF.Sin, bias=HPI)
            sy4 = act(at4[2], AF.Sin)
            cy4 = act(at4[2], AF.Sin, bias=HPI)
            ar.free(*at4)

            # ===== k4 =====
            vr4, dro4, sec4 = ln_exp_phase(ve4, cp4, wdz_s4)
            rd4, pd4, yd4 = att_dot(sr4, cr4, sp4, sec4, *om4)
            acc4 = thrust_acc(sr4, cr4, sp4, cp4, sy4, cy4, dro4, vr4)
            ar.free(sr4, cr4, sp4, cp4, sy4, cy4, sec4, dro4, *vr4)
            c4x, c4y = cross_xy(*om4)
            accumulate(ve4, 1.0, ACC_p)
            accumulate(acc4, 1.0, ACC_v)
            accumulate([rd4, pd4, yd4], 1.0, ACC_a)
            accumulate([c4x, c4y], 1.0, ACC_c)
            ar.free(*ve4, *om4, *acc4, rd4, pd4, yd4, c4x, c4y)
            ar.free(wdz_s2, wdz_s4, T_)

            # ================= final combine =================
            def out_pl(c):
                return OUT[:, c * W : (c + 1) * W]

            # pos: z needs -3*dt*G correction folded into ACC
            V.tensor_scalar(
                pl(ACC_p[2]), pl(ACC_p[2]), -3.0 * DT * G, None, OP.add
            )
            # vel: z needs -6*G
            V.tensor_scalar(pl(ACC_v[2]), pl(ACC_v[2]), -6.0 * G, None, OP.add)
            for j, (accs, base_c) in enumerate(
                [
                    (ACC_p, [PX, PY, PZ]),
                    (ACC_v, [VX, VY, VZ]),
                    (ACC_a, [AR, AP_, AY]),
                ]
            ):
                for i_ in range(3):
                    V.scalar_tensor_tensor(
                        out_pl(j * 3 + i_),
                        pl(accs[i_]),
                        DT6,
                        IN[base_c[i_]],
                        OP.mult,
                        OP.add,
                    )
            # out_om = (om + dt*dtq) -/+ dt6 * weighted cross sums
            V.scalar_tensor_tensor(
                out_pl(9), pl(ACC_c[0]), -DT6, pl(oxd4), OP.mult, OP.add
            )
            V.scalar_tensor_tensor(
                out_pl(10), pl(ACC_c[1]), DT6, pl(oyd4), OP.mult, OP.add
            )
            V.scalar_tensor_tensor(
                out_pl(11), pl(dtq[2]), DT, IN[OZ], OP.mult, OP.add
            )
            ar.free(*ACC_p, *ACC_v, *ACC_a, *ACC_c, *dtq, oxd4, oyd4)

            # ---- store ----
            for c in range(N_OUT):
                nc.sync.dma_start(
                    out=yout[c, :, t * W : (t + 1) * W],
                    in_=OUT[:, c * W : (c + 1) * W],
                )

    nc.compile()
    return nc


def _prep_gains(inputs):
    f = np.float32
    kp_pos = np.abs(np.asarray(inputs["kp_pos"], f))
    ki_pos = np.abs(np.asarray(inputs["ki_pos"], f))
    kp_vel = np.abs(np.asarray(inputs["kp_vel"], f))
    ki_vel = np.abs(np.asarray(inputs["ki_vel"], f))
    kd_vel = np.abs(np.asarray(inputs["kd_vel"], f))
    kp_att = np.abs(np.asarray(inputs["kp_att"], f))
    kp_rate = np.abs(np.asarray(inputs["kp_rate"], f))
    ki_rate = np.abs(np.asarray(inputs["ki_rate"], f))
    kd_rate = np.abs(np.asarray(inputs["kd_rate"], f))
    inv_dt = np.float32(1.0) / np.float32(DT)
    return {
        "kp": [float(x) for x in kp_pos],
        "kip": [float(x) for x in ki_pos],
        "c1v": [float(np.float32(kp_vel[j]) + np.float32(kd_vel[j]) * inv_dt) for j in range(3)],
        "c2v": [float(x) for x in ki_vel],
        "c3v": [float(np.float32(kd_vel[j]) * inv_dt) for j in range(3)],
        "katt": [float(x) for x in kp_att],
        "c1r": [float(np.float32(kp_rate[j]) + np.float32(kd_rate[j]) * inv_dt) for j in range(3)],
        "c2r": [float(x) for x in ki_rate],
        "c3r": [float(np.float32(kd_rate[j]) * inv_dt) for j in range(3)],
    }


def pack_inputs(inputs, F):
    """Pack full inputs into per-core [34, 128, F] SoA planes."""
    f = np.float32
    B = B_TOTAL
    R = P * F
    tot = N_CORES * R
    state = np.asarray(inputs["state"], f)
    ch = np.empty((N_IN, tot), f)
    ch[0:12, :B] = state.T
    ch[12:15, :B] = np.asarray(inputs["target_pos"], f).T
    ch[15, :B] = np.asarray(inputs["target_yaw"], f)
    ch[16:19, :B] = np.asarray(inputs["wind"], f).T
    ch[19:22, :B] = np.asarray(inputs["prev_vel_err"], f).T
    ch[22:25, :B] = np.asarray(inputs["prev_rate_err"], f).T
    ch[25:28, :B] = np.asarray(inputs["integral_pos"], f).T
    ch[28:31, :B] = np.asarray(inputs["integral_vel"], f).T
    ch[31:34, :B] = np.asarray(inputs["integral_rate"], f).T
    pad = tot - B
    if pad:
        ch[:, B:] = ch[:, :pad]
    return ch.reshape(N_IN, N_CORES, P, F).transpose(1, 0, 2, 3).copy()


def unpack_outputs(results, F):
    """results: list of per-core dicts with 'yout' [12, 128, F]."""
    y = np.stack([np.asarray(r["yout"]) for r in results])  # [8,12,P,F]
    y = y.transpose(1, 0, 2, 3).reshape(N_OUT, N_CORES * P * F)
    return np.ascontiguousarray(y[:, :B_TOTAL].T)


def kernel(**inputs):
    F = 1956
    W = 652
    gains = _prep_gains(inputs)
    X = pack_inputs(inputs, F)
    nc = build_nc(F, W, gains)
    in_maps = [{"xin": X[c]} for c in range(N_CORES)]
    res = run_bass_kernel_spmd(nc, in_maps, list(range(N_CORES)))
    return unpack_outputs(res.results, F)


# revision 58
# speedup vs baseline: 1.0150x; 1.0150x over previous
"""Trainium2 Bass kernel for the DifferentiableDroneController problem.

Strategy:
  - Pure data parallelism across 8 NeuronCores (batch split).
  - Host-side SoA repack: every per-row channel becomes a contiguous
    [128, F] plane on the device, so all device ops are dense
    elementwise plane ops (no strided access anywhere).
  - All transcendentals on the Scalar engine using only TWO activation
    table sets:
      * trig_and_small: Sin (cos via bias=pi/2), Arctan (arcsin via
        arctan(x*rsqrt(1-x^2)))
      * natural_log_exp_and_others: 1/x = exp(-ln x),
        rsqrt(x) = exp(-0.5 ln x), sqrt(x) = exp(0.5 ln x)
  - Vector engine uses fused ops: scalar_tensor_tensor (a*c + b in one
    instruction) and dual-op tensor_scalar (clip in one instruction).
  - Algebraic simplifications: I_VEC cancels out of omega_dot entirely,
    gravity is folded into constants, the acc-norm clamp is
    min(1, 10*rsqrt(s)).
"""

import math
from contextlib import ExitStack

import numpy as np

import concourse.bacc as bacc
import concourse.bass as bass
import concourse.mybir as mybir
from concourse import tile
from concourse.bass_utils import run_bass_kernel_spmd

AF = mybir.ActivationFunctionType
OP = mybir.AluOpType
F32 = mybir.dt.float32

P = 128
N_CORES = 8
B_TOTAL = 2_000_000
DT = 0.01
DT2 = 0.005
DT6 = DT / 6.0
G = 9.81
PI = math.pi
HPI = math.pi / 2.0
LN10 = math.log(10.0)
LN005 = math.log(0.05)

# channel indices in the packed input [34, P, F]
PX, PY, PZ = 0, 1, 2
VX, VY, VZ = 3, 4, 5
AR, AP_, AY = 6, 7, 8
OX, OY, OZ = 9, 10, 11
TPX, TPY, TPZ = 12, 13, 14
TYAW = 15
WDX, WDY, WDZ = 16, 17, 18
PVX, PVY, PVZ = 19, 20, 21
PRX, PRY, PRZ = 22, 23, 24
IPX, IPY, IPZ = 25, 26, 27
IVX, IVY, IVZ = 28, 29, 30
IRX, IRY, IRZ = 31, 32, 33

N_IN = 34
N_OUT = 12


class Arena:
    """Manual plane allocator over a single [P, NA*W] SBUF tile."""

    def __init__(self, ap, W, n_slots, n_reserved):
        self.ap = ap
        self.W = W
        self.free_list = list(range(n_reserved, n_slots))
        self.peak = n_reserved
        self.n_slots = n_slots
        self.live = n_reserved

    def plane(self, slot):
        W = self.W
        return self.ap[:, slot * W : (slot + 1) * W]

    def alloc(self):
        assert self.free_list, "arena exhausted"
        s = self.free_list.pop(0)
        self.live += 1
        self.peak = max(self.peak, self.n_slots - len(self.free_list))
        return s

    def free(self, *slots):
        for s in slots:
            assert s not in self.free_list
            self.free_list.append(s)
            self.live -= 1


def build_nc(F, W, gains, use_gpsimd=True):
    """Build the Bass program. gains: dict with kp(3), kip(3), c1v(3),
    c2v(3), c3v(3), katt(3), c1r(3), c2r(3), c3r(3) as python floats."""
    assert F % W == 0
    n_tiles = F // W

    kp = gains["kp"]
    kip = gains["kip"]
    c1v = gains["c1v"]
    c2v = gains["c2v"]
    c3v = gains["c3v"]
    katt = gains["katt"]
    c1r = gains["c1r"]
    c2r = gains["c2r"]
    c3r = gains["c3r"]

    NA = 66  # arena slots (measured peak liveness is 63)

    nc = bacc.Bacc()

    # The act-table-load inserter picks the FIRST table set containing a
    # function. By default that maps Ln -> natural_log (no exp) and
    # Exp -> exp_and_others (no ln), so every ln/exp pair costs two
    # ~1.7us table loads. Remove those functions from the suboptimal
    # sets in the cached table dict so both resolve to
    # natural_log_exp_and_others, and arctan to trig_and_small (shared
    # with sin). Indices of the remaining sets are unchanged, so the
    # emitted act_func_set_ids stay valid for walrus.
    from concourse.hw_specs import get_activation_tables

    tabs = get_activation_tables(nc.m.arch)
    tabs["sigmoid_and_others"].discard(AF.Arctan)

    # register const APs for the activation biases we use
    for cval in (HPI, G, DT2 * G, DT * G):
        cten = nc.alloc_sbuf_tensor(f"constu-f32-{cval}", [P, 1], F32)
        nc.gpsimd.memset(cten.ap(), cval)
        nc.const_aps.aps[(F32, cval)] = cten.ap()
    nc.all_engine_barrier()

    xin = nc.declare_dram_parameter("xin", [N_IN, P, F], F32, isOutput=False)
    yout = nc.declare_dram_parameter("yout", [N_OUT, P, F], F32, isOutput=True)

    with tile.TileContext(nc) as tc, ExitStack() as ctx:
        arena_pool = ctx.enter_context(tc.tile_pool(name="arena", bufs=1))
        out_pool = ctx.enter_context(tc.tile_pool(name="out", bufs=1))
        arena_tile = arena_pool.tile([P, NA * W], F32)

        V = nc.vector
        A = nc.scalar
        Gp = nc.gpsimd

        for t in range(n_tiles):
            ar = Arena(arena_tile[:], W, NA, N_IN)
            IN = [ar.plane(c) for c in range(N_IN)]

            # ---- load input planes (one DMA per channel keeps the
            # per-consumer sync-wait count within ISA limits) ----
            for c in range(N_IN):
                nc.sync.dma_start(
                    out=arena_tile[:, c * W : (c + 1) * W],
                    in_=xin[c, :, t * W : (t + 1) * W],
                )

            OUT = out_pool.tile([P, N_OUT * W], F32)

            def pl(s):
                return ar.plane(s)

            def new():
                return ar.alloc()

            # engine helpers
            def tt(op, a, b, out=None, eng=V):
                o = out if out is not None else new()
                eng.tensor_tensor(pl(o), pl(a), pl(b), op)
                return o

            def ts(a, s1, op0, s2=None, op1=None, out=None, eng=V):
                o = out if out is not None else new()
                if s2 is None:
                    eng.tensor_scalar(pl(o), pl(a), s1, None, op0)
                else:
                    eng.tensor_scalar(pl(o), pl(a), s1, s2, op0, op1)
                return o

            def stt(a, s, b, op0, op1, out=None, eng=V):
                o = out if out is not None else new()
                eng.scalar_tensor_tensor(pl(o), pl(a), s, pl(b), op0, op1)
                return o

            def act(a, func, bias=0.0, scale=1.0, out=None):
                o = out if out is not None else new()
                A.activation(pl(o), pl(a), func, bias=bias, scale=scale)
                return o

            def clip_ip(a, lo, hi):
                V.tensor_scalar(pl(a), pl(a), lo, hi, OP.max, OP.min)
                return a

            # ================= Phase T0: trig =================
            # target_yaw spans (-pi, pi): sin(x+pi/2) would leave the
            # HW sin domain [-pi, pi], so cos via 1 - 2*sin^2(y/2).
            sty = act(TYAW, AF.Sin)
            syh = act(TYAW, AF.Sin, scale=0.5)
            sy2q = act(syh, AF.Square)
            ar.free(syh)
            cty = ts(sy2q, -2.0, OP.mult, 1.0, OP.add)
            ar.free(sy2q)
            sr1 = act(AR, AF.Sin)
            cr1 = act(AR, AF.Sin, bias=HPI)
            sp1 = act(AP_, AF.Sin)
            cp1 = act(AP_, AF.Sin, bias=HPI)
            sy1 = act(AY, AF.Sin)
            cy1 = act(AY, AF.Sin, bias=HPI)

            # ================= controller part 1 (vector) =================
            dac = []
            for j, (pj, tpj, ipj, vj, ivj, pvj) in enumerate(
                [
                    (PX, TPX, IPX, VX, IVX, PVX),
                    (PY, TPY, IPY, VY, IVY, PVY),
                    (PZ, TPZ, IPZ, VZ, IVZ, PVZ),
                ]
            ):
                pe = tt(OP.subtract, tpj, pj)
                ip2 = stt(pe, DT, ipj, OP.mult, OP.add)
                clip_ip(ip2, -2.0, 2.0)
                q = act(pe, AF.Copy, scale=kp[j])
                ar.free(pe)
                dv = stt(ip2, kip[j], q, OP.mult, OP.add)
                ar.free(ip2, q)
                clip_ip(dv, -10.0, 10.0)
                ve = tt(OP.subtract, dv, vj)
                ar.free(dv)
                iv2 = stt(ve, DT, ivj, OP.mult, OP.add)
                clip_ip(iv2, -2.0, 2.0)
                q2 = act(ve, AF.Copy, scale=c1v[j])
                q3 = stt(iv2, c2v[j], q2, OP.mult, OP.add)
                ar.free(iv2, q2, ve)
                dacj = stt(pvj, -c3v[j], q3, OP.mult, OP.add)
                ar.free(q3)
                # controller-only input channels are dead now
                ar.free(tpj, ipj, ivj, pvj)
                dac.append(dacj)

            # v_rel for k1 (vector, independent)
            vr1 = [
                tt(OP.subtract, VX, WDX),
                tt(OP.subtract, VY, WDY),
                tt(OP.subtract, VZ, WDZ),
            ]
            # shifted wind-z for later stages (fold gravity)
            wdz_s2 = act(WDZ, AF.Identity, bias=DT2 * G)
            wdz_s4 = act(WDZ, AF.Identity, bias=DT * G)

            # ================= Phase L1: sqrt + reciprocal =========
            # acceleration-norm clamp factor = min(1, 10*rsqrt(s))
            sq0 = act(dac[0], AF.Square)
            sq1 = act(dac[1], AF.Square)
            sq2 = act(dac[2], AF.Square)
            sacc = tt(OP.add, sq0, sq1)
            tt(OP.add, sacc, sq2, out=sacc)
            ar.free(sq0, sq1, sq2)
            # sqrt(0.01*s) = 0.1*sqrt(s); recip -> 10/sqrt(s)
            racc = act(sacc, AF.Sqrt, scale=0.01)
            f10 = new()
            V.reciprocal(pl(f10), pl(racc))
            ar.free(sacc, racc)
            V.tensor_scalar(pl(f10), pl(f10), 1.0, None, OP.min)
            for j in range(3):
                tt(OP.mult, dac[j], f10, out=dac[j])
            ar.free(f10)
            # thrust vector: z gets +G (squares fold it via bias);
            # tn[:,2] is never used by the reference, so no explicit tvz.
            q0 = act(dac[0], AF.Square)
            q1 = act(dac[1], AF.Square)
            q2_ = act(dac[2], AF.Square, bias=G)
            s2 = tt(OP.add, q0, q1)
            tt(OP.add, s2, q2_, out=s2)
            ar.free(q0, q1, q2_)
            Tn = act(s2, AF.Sqrt)  # sqrt(s2) unclipped
            ar.free(s2)
            rsq2 = new()
            V.reciprocal(pl(rsq2), pl(Tn))
            T_ = ts(Tn, 0.1 * G, OP.max, 2.0 * G, OP.min)
            ar.free(Tn)
            tnx = tt(OP.mult, dac[0], rsq2)
            tny = tt(OP.mult, dac[1], rsq2)
            ar.free(dac[0], dac[1], dac[2], rsq2)
            # roll_arg
            ra = tt(OP.mult, tnx, sty)
            rb = tt(OP.mult, tny, cty)
            u = tt(OP.subtract, ra, rb)
            ar.free(ra, rb)
            # arcsin(u) = 2*arctan(u / (1 + sqrt(1-u^2))) keeps the
            # arctan argument inside the HW domain [-pi/2, pi/2].
            clip_ip(u, -0.999, 0.999)
            u2 = act(u, AF.Square)
            wu = act(u2, AF.Sqrt, bias=1.0, scale=-1.0)  # cos(des_roll)
            ar.free(u2)
            mru = new()
            V.reciprocal(pl(mru), pl(wu))  # 1/cos(des_roll)
            dnu = act(wu, AF.Identity, bias=1.0)
            ar.free(wu)
            rdu = new()
            V.reciprocal(pl(rdu), pl(dnu))
            ar.free(dnu)
            uarg = tt(OP.mult, u, rdu)
            ar.free(u, rdu)
            # pitch_arg
            pa = tt(OP.mult, tnx, cty)
            pb = tt(OP.mult, tny, sty)
            ar.free(tnx, tny, sty, cty)
            pc = tt(OP.add, pa, pb)
            v_ = tt(OP.mult, pc, mru)
            ar.free(pa, pb, pc, mru)
            clip_ip(v_, -0.999, 0.999)
            v2 = act(v_, AF.Square)
            wv = act(v2, AF.Sqrt, bias=1.0, scale=-1.0)
            ar.free(v2)
            dnv = act(wv, AF.Identity, bias=1.0)
            ar.free(wv)
            rdv = new()
            V.reciprocal(pl(rdv), pl(dnv))
            ar.free(dnv)
            varg = tt(OP.mult, v_, rdv)
            ar.free(v_, rdv)
            # k1 drag root and secant
            sv0 = act(vr1[0], AF.Square)
            sv1_ = act(vr1[1], AF.Square)
            sv2 = act(vr1[2], AF.Square)
            sv = tt(OP.add, sv0, sv1_)
            tt(OP.add, sv, sv2, out=sv)
            ar.free(sv0, sv1_, sv2)
            # 0.05*|v| = sqrt(0.0025*|v|^2)
            dro1 = act(sv, AF.Sqrt, scale=0.0025)
            ar.free(sv)
            sec1 = new()
            V.reciprocal(pl(sec1), pl(cp1))

            # ---------- dynamics helper (after trig + sec/dro ready) ----
            GE = Gp if use_gpsimd else V

            def att_dot(sr, cr, sp, sec, omx, omy, omz):
                m1 = tt(OP.mult, sr, omy, eng=GE)
                m2 = tt(OP.mult, cr, omz, eng=GE)
                m3 = tt(OP.add, m1, m2, eng=GE)
                ar.free(m1, m2)
                yd = tt(OP.mult, m3, sec, eng=GE)
                ar.free(m3)
                # roll_dot = wx + (sp*sec)*m3 = wx + sp*yd
                rda = tt(OP.mult, sp, yd, eng=GE)
                rd = tt(OP.add, rda, omx)
                ar.free(rda)
                pda = tt(OP.mult, cr, omy, eng=GE)
                pdb = tt(OP.mult, sr, omz, eng=GE)
                pd = tt(OP.subtract, pda, pdb, eng=GE)
                ar.free(pda, pdb)
                return rd, pd, yd

            def thrust_acc(sr, cr, sp, cp, sy, cy, dro, vr):
                t1 = tt(OP.mult, sp, cr, eng=GE)
                t2 = tt(OP.mult, cy, t1, eng=GE)
                t3 = tt(OP.mult, sy, sr, eng=GE)
                colx = tt(OP.add, t2, t3, eng=GE)
                ar.free(t2, t3)
                t4 = tt(OP.mult, sy, t1, eng=GE)
                t5 = tt(OP.mult, cy, sr, eng=GE)
                ar.free(t1)
                coly = tt(OP.subtract, t4, t5, eng=GE)
                ar.free(t4, t5)
                colz = tt(OP.mult, cp, cr, eng=GE)
                accs = []
                for colj, vrj in zip((colx, coly, colz), vr):
                    tg = tt(OP.mult, T_, colj, eng=GE)
                    ar.free(colj)
                    dr = tt(OP.mult, dro, vrj, eng=GE)
                    acc = tt(OP.subtract, tg, dr, eng=GE)
                    ar.free(tg, dr)
                    accs.append(acc)
                return accs

            def cross_xy(omx, omy, omz):
                # omega_dot_x = dtqx - cx, omega_dot_y = dtqy + cy;
                # the dtq part is hoisted into oxd2/oxd4 below, so only
                # the cross terms are computed per stage.
                cx = tt(OP.mult, omy, omz, eng=GE)
                cy_ = tt(OP.mult, omx, omz, eng=GE)
                return cx, cy_

            # k1 attitude dynamics (needs only state + sec1)
            rd1, pd1, yd1 = att_dot(sr1, cr1, sp1, sec1, OX, OY, OZ)
            # stage-2 attitude
            at2 = [
                stt(rd1, DT2, AR, OP.mult, OP.add),
                stt(pd1, DT2, AP_, OP.mult, OP.add),
                stt(yd1, DT2, AY, OP.mult, OP.add),
            ]

            # ================= Phase T2: trig =================
            droll = act(uarg, AF.Arctan)
            dpitch = act(varg, AF.Arctan)
            ar.free(uarg, varg)
            sr2 = act(at2[0], AF.Sin)
            cr2 = act(at2[0], AF.Sin, bias=HPI)
            sp2 = act(at2[1], AF.Sin)
            cp2 = act(at2[1], AF.Sin, bias=HPI)
            sy2 = act(at2[2], AF.Sin)
            cy2 = act(at2[2], AF.Sin, bias=HPI)
            ar.free(*at2)

            # ---- controller part 2 (vector) ----
            # droll/dpitch hold atan(tan(theta/2)); clip at 0.523/2 and
            # fold the *2 into the attitude-error subtraction.
            half_clip = float(np.float32(0.523) / np.float32(2.0))
            clip_ip(droll, -half_clip, half_clip)
            clip_ip(dpitch, -half_clip, half_clip)
            aer = stt(droll, 2.0, AR, OP.mult, OP.subtract)
            aep = stt(dpitch, 2.0, AP_, OP.mult, OP.subtract)
            ar.free(droll, dpitch)
            x_ = tt(OP.subtract, TYAW, AY)
            g1 = ts(x_, PI, OP.is_gt, 2.0 * PI, OP.mult)
            g2 = ts(x_, -PI, OP.is_lt, 2.0 * PI, OP.mult)
            x1 = stt(g1, -1.0, x_, OP.mult, OP.add)
            ar.free(g1, x_)
            aey = tt(OP.add, x1, g2)
            ar.free(x1, g2, TYAW)
            dtq = []
            for j, (aej, oj, irj, prj) in enumerate(
                [(aer, OX, IRX, PRX), (aep, OY, IRY, PRY), (aey, OZ, IRZ, PRZ)]
            ):
                re = stt(aej, katt[j], oj, OP.mult, OP.subtract)
                ar.free(aej)
                ir2 = stt(re, DT, irj, OP.mult, OP.add)
                clip_ip(ir2, -1.0, 1.0)
                q = act(re, AF.Copy, scale=c1r[j])
                q2 = stt(ir2, c2r[j], q, OP.mult, OP.add)
                ar.free(ir2, q, re)
                dtqj = stt(prj, -c3r[j], q2, OP.mult, OP.add)
                ar.free(q2, irj, prj)
                dtq.append(dtqj)

            # ---- finish k1 (vector) ----
            # hoisted omega + dt*dtq terms (shared by stages and final)
            oxd2 = stt(dtq[0], DT2, OX, OP.mult, OP.add)
            oyd2 = stt(dtq[1], DT2, OY, OP.mult, OP.add)
            acc1 = thrust_acc(sr1, cr1, sp1, cp1, sy1, cy1, dro1, vr1)
            ar.free(sr1, cr1, sp1, cp1, sy1, cy1, sec1, dro1, *vr1)
            c1x, c1y = cross_xy(OX, OY, OZ)
            # stage-2 velocity / omega
            ve2 = [
                stt(acc1[0], DT2, VX, OP.mult, OP.add),
                stt(acc1[1], DT2, VY, OP.mult, OP.add),
                stt(acc1[2], DT2, VZ, OP.mult, OP.add),
            ]
            om2 = [
                stt(c1x, -DT2, oxd2, OP.mult, OP.add),
                stt(c1y, DT2, oyd2, OP.mult, OP.add),
                stt(dtq[2], DT2, OZ, OP.mult, OP.add),
            ]

            # generic stage: given trig phase done for atI, compute
            # dynamics kI, accumulate, produce next stage state.
            ACC_p = [None, None, None]
            ACC_v = [None, None, None]
            ACC_a = [None, None, None]
            ACC_c = [None, None]  # weighted cross-term sums

            def ln_exp_phase(veI, atI_trig_cp, vrz_shift):
                """v_rel, drag root, secant for one stage."""
                vrI = [
                    tt(OP.subtract, veI[0], WDX, eng=GE),
                    tt(OP.subtract, veI[1], WDY, eng=GE),
                    tt(OP.subtract, veI[2], vrz_shift, eng=GE),
                ]
                s0 = act(vrI[0], AF.Square)
                s1 = act(vrI[1], AF.Square)
                s2_ = act(vrI[2], AF.Square)
                sv_ = tt(OP.add, s0, s1)
                tt(OP.add, sv_, s2_, out=sv_)
                ar.free(s0, s1, s2_)
                dro = act(sv_, AF.Sqrt, scale=0.0025)
                ar.free(sv_)
                sec = new()
                V.reciprocal(pl(sec), pl(atI_trig_cp))
                return vrI, dro, sec

            def accumulate(planes, weight, slot_list, base=None, eng=V):
                """ACC update: ACC = base + weight*planes (init) or
                ACC += weight*planes."""
                for i_, p_ in enumerate(planes):
                    if slot_list[i_] is None:
                        # init: ACC = weight*p + base_i
                        slot_list[i_] = stt(
                            p_, weight, base[i_], OP.mult, OP.add, eng=eng
                        )
                    else:
                        if weight == 1.0:
                            tt(
                                OP.add,
                                slot_list[i_],
                                p_,
                                out=slot_list[i_],
                                eng=eng,
                            )
                        else:
                            stt(
                                p_,
                                weight,
                                slot_list[i_],
                                OP.mult,
                                OP.add,
                                out=slot_list[i_],
                                eng=eng,
                            )

            # ===== k2 =====
            vr2, dro2, sec2 = ln_exp_phase(ve2, cp2, wdz_s2)
            rd2, pd2, yd2 = att_dot(sr2, cr2, sp2, sec2, *om2)
            acc2 = thrust_acc(sr2, cr2, sp2, cp2, sy2, cy2, dro2, vr2)
            ar.free(sr2, cr2, sp2, cp2, sy2, cy2, sec2, dro2, *vr2)
            c2x, c2y = cross_xy(*om2)
            # stage-3 state
            at3 = [
                stt(rd2, DT2, AR, OP.mult, OP.add),
                stt(pd2, DT2, AP_, OP.mult, OP.add),
                stt(yd2, DT2, AY, OP.mult, OP.add),
            ]
            ve3 = [
                stt(acc2[0], DT2, VX, OP.mult, OP.add),
                stt(acc2[1], DT2, VY, OP.mult, OP.add),
                stt(acc2[2], DT2, VZ, OP.mult, OP.add),
            ]
            om3 = [
                stt(c2x, -DT2, oxd2, OP.mult, OP.add),
                stt(c2y, DT2, oyd2, OP.mult, OP.add),
                stt(dtq[2], DT2, OZ, OP.mult, OP.add),
            ]
            # ACC init with k1 + 2*k2
            accumulate(ve2, 2.0, ACC_p, base=[VX, VY, VZ])
            accumulate(acc2, 2.0, ACC_v, base=acc1)
            accumulate([rd2, pd2, yd2], 2.0, ACC_a, base=[rd1, pd1, yd1])
            accumulate([c2x, c2y], 2.0, ACC_c, base=[c1x, c1y])
            ar.free(*ve2, *om2, *acc1, *acc2, rd1, pd1, yd1, rd2, pd2, yd2)
            ar.free(c1x, c1y, c2x, c2y, oxd2, oyd2)

            # ================= Phase T4: trig =================
            sr3 = act(at3[0], AF.Sin)
            cr3 = act(at3[0], AF.Sin, bias=HPI)
            sp3 = act(at3[1], AF.Sin)
            cp3 = act(at3[1], AF.Sin, bias=HPI)
            sy3 = act(at3[2], AF.Sin)
            cy3 = act(at3[2], AF.Sin, bias=HPI)
            ar.free(*at3)

            # ===== k3 =====
            vr3, dro3, sec3 = ln_exp_phase(ve3, cp3, wdz_s2)
            rd3, pd3, yd3 = att_dot(sr3, cr3, sp3, sec3, *om3)
            acc3 = thrust_acc(sr3, cr3, sp3, cp3, sy3, cy3, dro3, vr3)
            ar.free(sr3, cr3, sp3, cp3, sy3, cy3, sec3, dro3, *vr3)
            c3x, c3y = cross_xy(*om3)
            oxd4 = stt(dtq[0], DT, OX, OP.mult, OP.add)
            oyd4 = stt(dtq[1], DT, OY, OP.mult, OP.add)
            # stage-4 state (full dt)
            at4 = [
                stt(rd3, DT, AR, OP.mult, OP.add),
                stt(pd3, DT, AP_, OP.mult, OP.add),
                stt(yd3, DT, AY, OP.mult, OP.add),
            ]
            ve4 = [
                stt(acc3[0], DT, VX, OP.mult, OP.add),
                stt(acc3[1], DT, VY, OP.mult, OP.add),
                stt(acc3[2], DT, VZ, OP.mult, OP.add),
            ]
            om4 = [
                stt(c3x, -DT, oxd4, OP.mult, OP.add),
                stt(c3y, DT, oyd4, OP.mult, OP.add),
                stt(dtq[2], DT, OZ, OP.mult, OP.add),
            ]
            accumulate(ve3, 2.0, ACC_p)
            accumulate(acc3, 2.0, ACC_v)
            accumulate([rd3, pd3, yd3], 2.0, ACC_a)
            accumulate([c3x, c3y], 2.0, ACC_c)
            ar.free(*ve3, *om3, *acc3, rd3, pd3, yd3, c3x, c3y)

            # ================= Phase T6: trig =================
            sr4 = act(at4[0], AF.Sin)
            cr4 = act(at4[0], AF.Sin, bias=HPI)
            sp4 = act(at4[1], AF.Sin)
            cp4 = act(at4[1], AF.Sin, bias=HPI)
            sy4 = act(at4[2], AF.Sin)
            cy4 = act(at4[2], AF.Sin, bias=HPI)
            ar.free(*at4)

            # ===== k4 =====
            vr4, dro4, sec4 = ln_exp_phase(ve4, cp4, wdz_s4)
            rd4, pd4, yd4 = att_dot(sr4, cr4, sp4, sec4, *om4)
            acc4 = thrust_acc(sr4, cr4, sp4, cp4, sy4, cy4, dro4, vr4)
            ar.free(sr4, cr4, sp4, cp4, sy4, cy4, sec4, dro4, *vr4)
            c4x, c4y = cross_xy(*om4)
            accumulate(ve4, 1.0, ACC_p, eng=GE)
            accumulate(acc4, 1.0, ACC_v)
            accumulate([rd4, pd4, yd4], 1.0, ACC_a, eng=GE)
            accumulate([c4x, c4y], 1.0, ACC_c)
            ar.free(*ve4, *om4, *acc4, rd4, pd4, yd4, c4x, c4y)
            ar.free(wdz_s2, wdz_s4, T_)

            # ================= final combine =================
            def out_pl(c):
                return OUT[:, c * W : (c + 1) * W]

            # pos: z needs -3*dt*G correction folded into ACC
            V.tensor_scalar(
                pl(ACC_p[2]), pl(ACC_p[2]), -3.0 * DT * G, None, OP.add
            )
            # vel: z needs -6*G
            V.tensor_scalar(pl(ACC_v[2]), pl(ACC_v[2]), -6.0 * G, None, OP.add)
            # split the tail across DVE and Pool so the final combine
            # doesn't serialize on one engine before the out-DMA
            for j, (accs, base_c, eng_) in enumerate(
                [
                    (ACC_p, [PX, PY, PZ], V),
                    (ACC_v, [VX, VY, VZ], V),
                    (ACC_a, [AR, AP_, AY], V),
                ]
            ):
                for i_ in range(3):
                    eng_.scalar_tensor_tensor(
                        out_pl(j * 3 + i_),
                        pl(accs[i_]),
                        DT6,
                        IN[base_c[i_]],
                        OP.mult,
                        OP.add,
                    )
            # out_om = (om + dt*dtq) -/+ dt6 * weighted cross sums
            V.scalar_tensor_tensor(
                out_pl(9), pl(ACC_c[0]), -DT6, pl(oxd4), OP.mult, OP.add
            )
            V.scalar_tensor_tensor(
                out_pl(10), pl(ACC_c[1]), DT6, pl(oyd4), OP.mult, OP.add
            )
            V.scalar_tensor_tensor(
                out_pl(11), pl(dtq[2]), DT, IN[OZ], OP.mult, OP.add
            )
            ar.free(*ACC_p, *ACC_v, *ACC_a, *ACC_c, *dtq, oxd4, oyd4)

            # ---- store ----
            for c in range(N_OUT):
                nc.sync.dma_start(
                    out=yout[c, :, t * W : (t + 1) * W],
                    in_=OUT[:, c * W : (c + 1) * W],
                )

    nc.compile()
    return nc


def _prep_gains(inputs):
    f = np.float32
    kp_pos = np.abs(np.asarray(inputs["kp_pos"], f))
    ki_pos = np.abs(np.asarray(inputs["ki_pos"], f))
    kp_vel = np.abs(np.asarray(inputs["kp_vel"], f))
    ki_vel = np.abs(np.asarray(inputs["ki_vel"], f))
    kd_vel = np.abs(np.asarray(inputs["kd_vel"], f))
    kp_att = np.abs(np.asarray(inputs["kp_att"], f))
    kp_rate = np.abs(np.asarray(inputs["kp_rate"], f))
    ki_rate = np.abs(np.asarray(inputs["ki_rate"], f))
    kd_rate = np.abs(np.asarray(inputs["kd_rate"], f))
    inv_dt = np.float32(1.0) / np.float32(DT)
    return {
        "kp": [float(x) for x in kp_pos],
        "kip": [float(x) for x in ki_pos],
        "c1v": [float(np.float32(kp_vel[j]) + np.float32(kd_vel[j]) * inv_dt) for j in range(3)],
        "c2v": [float(x) for x in ki_vel],
        "c3v": [float(np.float32(kd_vel[j]) * inv_dt) for j in range(3)],
        "katt": [float(x) for x in kp_att],
        "c1r": [float(np.float32(kp_rate[j]) + np.float32(kd_rate[j]) * inv_dt) for j in range(3)],
        "c2r": [float(x) for x in ki_rate],
        "c3r": [float(np.float32(kd_rate[j]) * inv_dt) for j in range(3)],
    }


def pack_inputs(inputs, F):
    """Pack full inputs into per-core [34, 128, F] SoA planes."""
    f = np.float32
    B = B_TOTAL
    R = P * F
    tot = N_CORES * R
    state = np.asarray(inputs["state"], f)
    ch = np.empty((N_IN, tot), f)
    ch[0:12, :B] = state.T
    ch[12:15, :B] = np.asarray(inputs["target_pos"], f).T
    ch[15, :B] = np.asarray(inputs["target_yaw"], f)
    ch[16:19, :B] = np.asarray(inputs["wind"], f).T
    ch[19:22, :B] = np.asarray(inputs["prev_vel_err"], f).T
    ch[22:25, :B] = np.asarray(inputs["prev_rate_err"], f).T
    ch[25:28, :B] = np.asarray(inputs["integral_pos"], f).T
    ch[28:31, :B] = np.asarray(inputs["integral_vel"], f).T
    ch[31:34, :B] = np.asarray(inputs["integral_rate"], f).T
    pad = tot - B
    if pad:
        ch[:, B:] = ch[:, :pad]
    return ch.reshape(N_IN, N_CORES, P, F).transpose(1, 0, 2, 3).copy()


def unpack_outputs(results, F):
    """results: list of per-core dicts with 'yout' [12, 128, F]."""
    y = np.stack([np.asarray(r["yout"]) for r in results])  # [8,12,P,F]
    y = y.transpose(1, 0, 2, 3).reshape(N_OUT, N_CORES * P * F)
    return np.ascontiguousarray(y[:, :B_TOTAL].T)


def kernel(**inputs):
    F = 1956
    W = 652
    gains = _prep_gains(inputs)
    X = pack_inputs(inputs, F)
    nc = build_nc(F, W, gains)
    in_maps = [{"xin": X[c]} for c in range(N_CORES)]
    res = run_bass_kernel_spmd(nc, in_maps, list(range(N_CORES)))
    return unpack_outputs(res.results, F)


# revision 63
# speedup vs baseline: 1.1293x; 1.1125x over previous
"""Trainium2 Bass kernel for the DifferentiableDroneController problem.

Strategy:
  - Pure data parallelism across 8 NeuronCores (batch split).
  - Host-side SoA repack: every per-row channel becomes a contiguous
    [128, F] plane on the device, so all device ops are dense
    elementwise plane ops (no strided access anywhere).
  - All transcendentals on the Scalar engine using only TWO activation
    table sets:
      * trig_and_small: Sin (cos via bias=pi/2), Arctan (arcsin via
        arctan(x*rsqrt(1-x^2)))
      * natural_log_exp_and_others: 1/x = exp(-ln x),
        rsqrt(x) = exp(-0.5 ln x), sqrt(x) = exp(0.5 ln x)
  - Vector engine uses fused ops: scalar_tensor_tensor (a*c + b in one
    instruction) and dual-op tensor_scalar (clip in one instruction).
  - Algebraic simplifications: I_VEC cancels out of omega_dot entirely,
    gravity is folded into constants, the acc-norm clamp is
    min(1, 10*rsqrt(s)).
"""

import math
from contextlib import ExitStack

import numpy as np

import concourse.bacc as bacc
import concourse.bass as bass
import concourse.mybir as mybir
from concourse import tile
from concourse.bass_utils import run_bass_kernel_spmd

AF = mybir.ActivationFunctionType
OP = mybir.AluOpType
F32 = mybir.dt.float32

P = 128
N_CORES = 8
B_TOTAL = 2_000_000
DT = 0.01
DT2 = 0.005
DT6 = DT / 6.0
G = 9.81
PI = math.pi
HPI = math.pi / 2.0
LN10 = math.log(10.0)
LN005 = math.log(0.05)

# channel indices in the packed input [34, P, F]
PX, PY, PZ = 0, 1, 2
VX, VY, VZ = 3, 4, 5
AR, AP_, AY = 6, 7, 8
OX, OY, OZ = 9, 10, 11
TPX, TPY, TPZ = 12, 13, 14
TYAW = 15
WDX, WDY, WDZ = 16, 17, 18
PVX, PVY, PVZ = 19, 20, 21
PRX, PRY, PRZ = 22, 23, 24
IPX, IPY, IPZ = 25, 26, 27
IVX, IVY, IVZ = 28, 29, 30
IRX, IRY, IRZ = 31, 32, 33

N_IN = 34
N_OUT = 12


class Arena:
    """Manual plane allocator over a single [P, NA*W] SBUF tile."""

    def __init__(self, ap, W, n_slots, n_reserved):
        self.ap = ap
        self.W = W
        self.free_list = list(range(n_reserved, n_slots))
        self.peak = n_reserved
        self.n_slots = n_slots
        self.live = n_reserved

    def plane(self, slot):
        W = self.W
        return self.ap[:, slot * W : (slot + 1) * W]

    def alloc(self):
        assert self.free_list, "arena exhausted"
        s = self.free_list.pop(0)
        self.live += 1
        self.peak = max(self.peak, self.n_slots - len(self.free_list))
        return s

    def free(self, *slots):
        for s in slots:
            assert s not in self.free_list
            self.free_list.append(s)
            self.live -= 1


def build_nc(F, W, gains, use_gpsimd=True):
    """Build the Bass program. gains: dict with kp(3), kip(3), c1v(3),
    c2v(3), c3v(3), katt(3), c1r(3), c2r(3), c3r(3) as python floats."""
    assert F % W == 0
    n_tiles = F // W

    kp = gains["kp"]
    kip = gains["kip"]
    c1v = gains["c1v"]
    c2v = gains["c2v"]
    c3v = gains["c3v"]
    katt = gains["katt"]
    c1r = gains["c1r"]
    c2r = gains["c2r"]
    c3r = gains["c3r"]

    NA = 66  # arena slots (measured peak liveness is 63)

    nc = bacc.Bacc()

    # The act-table-load inserter picks the FIRST table set containing a
    # function. By default that maps Ln -> natural_log (no exp) and
    # Exp -> exp_and_others (no ln), so every ln/exp pair costs two
    # ~1.7us table loads. Remove those functions from the suboptimal
    # sets in the cached table dict so both resolve to
    # natural_log_exp_and_others, and arctan to trig_and_small (shared
    # with sin). Indices of the remaining sets are unchanged, so the
    # emitted act_func_set_ids stay valid for walrus.
    from concourse.hw_specs import get_activation_tables

    tabs = get_activation_tables(nc.m.arch)
    tabs["sigmoid_and_others"].discard(AF.Arctan)

    # register const APs for the activation biases we use
    for cval in (HPI, G, DT2 * G, DT * G):
        cten = nc.alloc_sbuf_tensor(f"constu-f32-{cval}", [P, 1], F32)
        nc.gpsimd.memset(cten.ap(), cval)
        nc.const_aps.aps[(F32, cval)] = cten.ap()
    nc.all_engine_barrier()

    xin = nc.declare_dram_parameter("xin", [N_IN, P, F], F32, isOutput=False)
    yout = nc.declare_dram_parameter("yout", [N_OUT, P, F], F32, isOutput=True)

    with tile.TileContext(nc) as tc, ExitStack() as ctx:
        arena_pool = ctx.enter_context(tc.tile_pool(name="arena", bufs=1))
        out_pool = ctx.enter_context(tc.tile_pool(name="out", bufs=1))
        arena_tile = arena_pool.tile([P, NA * W], F32)

        V = nc.vector
        A = nc.scalar
        Gp = nc.gpsimd

        for t in range(n_tiles):
            ar = Arena(arena_tile[:], W, NA, N_IN)
            IN = [ar.plane(c) for c in range(N_IN)]

            # ---- load input planes (one DMA per channel keeps the
            # per-consumer sync-wait count within ISA limits); ordered
            # by first use so compute starts as early as possible ----
            dma_order = (
                [TYAW, AR, AP_, AY]
                + [TPX, TPY, TPZ, PX, PY, PZ]
                + [IPX, IPY, IPZ, VX, VY, VZ]
                + [IVX, IVY, IVZ, PVX, PVY, PVZ]
                + [OX, OY, OZ, IRX, IRY, IRZ]
                + [PRX, PRY, PRZ, WDX, WDY, WDZ]
            )
            for c in dma_order:
                nc.sync.dma_start(
                    out=arena_tile[:, c * W : (c + 1) * W],
                    in_=xin[c, :, t * W : (t + 1) * W],
                )

            OUT = out_pool.tile([P, N_OUT * W], F32)

            def pl(s):
                return ar.plane(s)

            def new():
                return ar.alloc()

            # engine helpers
            def tt(op, a, b, out=None, eng=V):
                o = out if out is not None else new()
                eng.tensor_tensor(pl(o), pl(a), pl(b), op)
                return o

            def ts(a, s1, op0, s2=None, op1=None, out=None, eng=V):
                o = out if out is not None else new()
                if s2 is None:
                    eng.tensor_scalar(pl(o), pl(a), s1, None, op0)
                else:
                    eng.tensor_scalar(pl(o), pl(a), s1, s2, op0, op1)
                return o

            def stt(a, s, b, op0, op1, out=None, eng=V):
                o = out if out is not None else new()
                eng.scalar_tensor_tensor(pl(o), pl(a), s, pl(b), op0, op1)
                return o

            def act(a, func, bias=0.0, scale=1.0, out=None):
                o = out if out is not None else new()
                A.activation(pl(o), pl(a), func, bias=bias, scale=scale)
                return o

            def clip_ip(a, lo, hi):
                V.tensor_scalar(pl(a), pl(a), lo, hi, OP.max, OP.min)
                return a

            # ================= Phase T0: trig =================
            # target_yaw spans (-pi, pi): sin(x+pi/2) would leave the
            # HW sin domain [-pi, pi], so cos via 1 - 2*sin^2(y/2).
            sty = act(TYAW, AF.Sin)
            syh = act(TYAW, AF.Sin, scale=0.5)
            sy2q = act(syh, AF.Square)
            ar.free(syh)
            cty = ts(sy2q, -2.0, OP.mult, 1.0, OP.add)
            ar.free(sy2q)
            sr1 = act(AR, AF.Sin)
            cr1 = act(AR, AF.Sin, bias=HPI)
            sp1 = act(AP_, AF.Sin)
            cp1 = act(AP_, AF.Sin, bias=HPI)
            sy1 = act(AY, AF.Sin)
            cy1 = act(AY, AF.Sin, bias=HPI)

            # ================= controller part 1 (vector) =================
            dac = []
            for j, (pj, tpj, ipj, vj, ivj, pvj) in enumerate(
                [
                    (PX, TPX, IPX, VX, IVX, PVX),
                    (PY, TPY, IPY, VY, IVY, PVY),
                    (PZ, TPZ, IPZ, VZ, IVZ, PVZ),
                ]
            ):
                pe = tt(OP.subtract, tpj, pj)
                ip2 = stt(pe, DT, ipj, OP.mult, OP.add)
                clip_ip(ip2, -2.0, 2.0)
                q = act(pe, AF.Copy, scale=kp[j])
                ar.free(pe)
                dv = stt(ip2, kip[j], q, OP.mult, OP.add)
                ar.free(ip2, q)
                clip_ip(dv, -10.0, 10.0)
                ve = tt(OP.subtract, dv, vj)
                ar.free(dv)
                iv2 = stt(ve, DT, ivj, OP.mult, OP.add)
                clip_ip(iv2, -2.0, 2.0)
                q2 = act(ve, AF.Copy, scale=c1v[j])
                q3 = stt(iv2, c2v[j], q2, OP.mult, OP.add)
                ar.free(iv2, q2, ve)
                dacj = stt(pvj, -c3v[j], q3, OP.mult, OP.add)
                ar.free(q3)
                # controller-only input channels are dead now
                ar.free(tpj, ipj, ivj, pvj)
                dac.append(dacj)

            # v_rel for k1 (vector, independent)
            vr1 = [
                tt(OP.subtract, VX, WDX),
                tt(OP.subtract, VY, WDY),
                tt(OP.subtract, VZ, WDZ),
            ]
            # shifted wind-z for later stages (fold gravity); scratch
            # copies of wind-x/y release the input slots early so the
            # next tile's wind DMAs overlap this tile's dynamics
            wdz_s2 = act(WDZ, AF.Identity, bias=DT2 * G)
            wdz_s4 = act(WDZ, AF.Identity, bias=DT * G)
            wdx_c = act(WDX, AF.Copy)
            wdy_c = act(WDY, AF.Copy)

            # ================= Phase L1: sqrt + reciprocal =========
            # acceleration-norm clamp factor = min(1, 10*rsqrt(s))
            sq0 = act(dac[0], AF.Square)
            sq1 = act(dac[1], AF.Square)
            sq2 = act(dac[2], AF.Square)
            sacc = tt(OP.add, sq0, sq1)
            tt(OP.add, sacc, sq2, out=sacc)
            ar.free(sq0, sq1, sq2)
            # sqrt(0.01*s) = 0.1*sqrt(s); recip -> 10/sqrt(s)
            racc = act(sacc, AF.Sqrt, scale=0.01)
            f10 = new()
            V.reciprocal(pl(f10), pl(racc))
            ar.free(sacc, racc)
            V.tensor_scalar(pl(f10), pl(f10), 1.0, None, OP.min)
            for j in range(3):
                tt(OP.mult, dac[j], f10, out=dac[j])
            ar.free(f10)
            # thrust vector: z gets +G (squares fold it via bias);
            # tn[:,2] is never used by the reference, so no explicit tvz.
            q0 = act(dac[0], AF.Square)
            q1 = act(dac[1], AF.Square)
            q2_ = act(dac[2], AF.Square, bias=G)
            s2 = tt(OP.add, q0, q1)
            tt(OP.add, s2, q2_, out=s2)
            ar.free(q0, q1, q2_)
            Tn = act(s2, AF.Sqrt)  # sqrt(s2) unclipped
            ar.free(s2)
            rsq2 = new()
            V.reciprocal(pl(rsq2), pl(Tn))
            T_ = ts(Tn, 0.1 * G, OP.max, 2.0 * G, OP.min)
            ar.free(Tn)
            tnx = tt(OP.mult, dac[0], rsq2)
            tny = tt(OP.mult, dac[1], rsq2)
            ar.free(dac[0], dac[1], dac[2], rsq2)
            # roll_arg
            ra = tt(OP.mult, tnx, sty)
            rb = tt(OP.mult, tny, cty)
            u = tt(OP.subtract, ra, rb)
            ar.free(ra, rb)
            # arcsin(u) = 2*arctan(u / (1 + sqrt(1-u^2))) keeps the
            # arctan argument inside the HW domain [-pi/2, pi/2].
            clip_ip(u, -0.999, 0.999)
            u2 = act(u, AF.Square)
            wu = act(u2, AF.Sqrt, bias=1.0, scale=-1.0)  # cos(des_roll)
            ar.free(u2)
            mru = new()
            V.reciprocal(pl(mru), pl(wu))  # 1/cos(des_roll)
            dnu = act(wu, AF.Identity, bias=1.0)
            ar.free(wu)
            rdu = new()
            V.reciprocal(pl(rdu), pl(dnu))
            ar.free(dnu)
            uarg = tt(OP.mult, u, rdu)
            ar.free(u, rdu)
            # pitch_arg
            pa = tt(OP.mult, tnx, cty)
            pb = tt(OP.mult, tny, sty)
            ar.free(tnx, tny, sty, cty)
            pc = tt(OP.add, pa, pb)
            v_ = tt(OP.mult, pc, mru)
            ar.free(pa, pb, pc, mru)
            clip_ip(v_, -0.999, 0.999)
            v2 = act(v_, AF.Square)
            wv = act(v2, AF.Sqrt, bias=1.0, scale=-1.0)
            ar.free(v2)
            dnv = act(wv, AF.Identity, bias=1.0)
            ar.free(wv)
            rdv = new()
            V.reciprocal(pl(rdv), pl(dnv))
            ar.free(dnv)
            varg = tt(OP.mult, v_, rdv)
            ar.free(v_, rdv)
            # k1 drag root and secant
            sv0 = act(vr1[0], AF.Square)
            sv1_ = act(vr1[1], AF.Square)
            sv2 = act(vr1[2], AF.Square)
            sv = tt(OP.add, sv0, sv1_)
            tt(OP.add, sv, sv2, out=sv)
            ar.free(sv0, sv1_, sv2)
            # 0.05*|v| = sqrt(0.0025*|v|^2)
            dro1 = act(sv, AF.Sqrt, scale=0.0025)
            ar.free(sv)
            sec1 = new()
            V.reciprocal(pl(sec1), pl(cp1))

            # ---------- dynamics helper (after trig + sec/dro ready) ----
            GE = Gp if use_gpsimd else V

            def att_dot(sr, cr, sp, sec, omx, omy, omz):
                m1 = tt(OP.mult, sr, omy, eng=GE)
                m2 = tt(OP.mult, cr, omz, eng=GE)
                m3 = tt(OP.add, m1, m2, eng=GE)
                ar.free(m1, m2)
                yd = tt(OP.mult, m3, sec, eng=GE)
                ar.free(m3)
                # roll_dot = wx + (sp*sec)*m3 = wx + sp*yd
                rda = tt(OP.mult, sp, yd, eng=GE)
                rd = tt(OP.add, rda, omx)
                ar.free(rda)
                pda = tt(OP.mult, cr, omy, eng=GE)
                pdb = tt(OP.mult, sr, omz, eng=GE)
                pd = tt(OP.subtract, pda, pdb, eng=GE)
                ar.free(pda, pdb)
                return rd, pd, yd

            def thrust_acc(sr, cr, sp, cp, sy, cy, dro, vr):
                t1 = tt(OP.mult, sp, cr, eng=GE)
                t2 = tt(OP.mult, cy, t1, eng=GE)
                t3 = tt(OP.mult, sy, sr, eng=GE)
                colx = tt(OP.add, t2, t3, eng=GE)
                ar.free(t2, t3)
                t4 = tt(OP.mult, sy, t1, eng=GE)
                t5 = tt(OP.mult, cy, sr, eng=GE)
                ar.free(t1)
                coly = tt(OP.subtract, t4, t5, eng=GE)
                ar.free(t4, t5)
                colz = tt(OP.mult, cp, cr, eng=GE)
                accs = []
                for colj, vrj in zip((colx, coly, colz), vr):
                    tg = tt(OP.mult, T_, colj, eng=GE)
                    ar.free(colj)
                    dr = tt(OP.mult, dro, vrj, eng=GE)
                    acc = tt(OP.subtract, tg, dr, eng=GE)
                    ar.free(tg, dr)
                    accs.append(acc)
                return accs

            def cross_xy(omx, omy, omz):
                # omega_dot_x = dtqx - cx, omega_dot_y = dtqy + cy;
                # the dtq part is hoisted into oxd2/oxd4 below, so only
                # the cross terms are computed per stage.
                cx = tt(OP.mult, omy, omz, eng=GE)
                cy_ = tt(OP.mult, omx, omz, eng=GE)
                return cx, cy_

            # k1 attitude dynamics (needs only state + sec1)
            rd1, pd1, yd1 = att_dot(sr1, cr1, sp1, sec1, OX, OY, OZ)
            # stage-2 attitude
            at2 = [
                stt(rd1, DT2, AR, OP.mult, OP.add),
                stt(pd1, DT2, AP_, OP.mult, OP.add),
                stt(yd1, DT2, AY, OP.mult, OP.add),
            ]

            # ================= Phase T2: trig =================
            droll = act(uarg, AF.Arctan)
            dpitch = act(varg, AF.Arctan)
            ar.free(uarg, varg)
            sr2 = act(at2[0], AF.Sin)
            cr2 = act(at2[0], AF.Sin, bias=HPI)
            sp2 = act(at2[1], AF.Sin)
            cp2 = act(at2[1], AF.Sin, bias=HPI)
            sy2 = act(at2[2], AF.Sin)
            cy2 = act(at2[2], AF.Sin, bias=HPI)
            ar.free(*at2)

            # ---- controller part 2 (vector) ----
            # droll/dpitch hold atan(tan(theta/2)); clip at 0.523/2 and
            # fold the *2 into the attitude-error subtraction.
            half_clip = float(np.float32(0.523) / np.float32(2.0))
            clip_ip(droll, -half_clip, half_clip)
            clip_ip(dpitch, -half_clip, half_clip)
            aer = stt(droll, 2.0, AR, OP.mult, OP.subtract)
            aep = stt(dpitch, 2.0, AP_, OP.mult, OP.subtract)
            ar.free(droll, dpitch)
            x_ = tt(OP.subtract, TYAW, AY)
            g1 = ts(x_, PI, OP.is_gt, 2.0 * PI, OP.mult)
            g2 = ts(x_, -PI, OP.is_lt, 2.0 * PI, OP.mult)
            x1 = stt(g1, -1.0, x_, OP.mult, OP.add)
            ar.free(g1, x_)
            aey = tt(OP.add, x1, g2)
            ar.free(x1, g2, TYAW)
            dtq = []
            for j, (aej, oj, irj, prj) in enumerate(
                [(aer, OX, IRX, PRX), (aep, OY, IRY, PRY), (aey, OZ, IRZ, PRZ)]
            ):
                re = stt(aej, katt[j], oj, OP.mult, OP.subtract)
                ar.free(aej)
                ir2 = stt(re, DT, irj, OP.mult, OP.add)
                clip_ip(ir2, -1.0, 1.0)
                q = act(re, AF.Copy, scale=c1r[j])
                q2 = stt(ir2, c2r[j], q, OP.mult, OP.add)
                ar.free(ir2, q, re)
                dtqj = stt(prj, -c3r[j], q2, OP.mult, OP.add)
                ar.free(q2, irj, prj)
                dtq.append(dtqj)

            # ---- finish k1 (vector) ----
            # hoisted omega + dt*dtq terms (shared by stages and final)
            oxd2 = stt(dtq[0], DT2, OX, OP.mult, OP.add)
            oyd2 = stt(dtq[1], DT2, OY, OP.mult, OP.add)
            acc1 = thrust_acc(sr1, cr1, sp1, cp1, sy1, cy1, dro1, vr1)
            ar.free(sr1, cr1, sp1, cp1, sy1, cy1, sec1, dro1, *vr1)
            c1x, c1y = cross_xy(OX, OY, OZ)
            # stage-2 velocity / omega
            ve2 = [
                stt(acc1[0], DT2, VX, OP.mult, OP.add),
                stt(acc1[1], DT2, VY, OP.mult, OP.add),
                stt(acc1[2], DT2, VZ, OP.mult, OP.add),
            ]
            om2 = [
                stt(c1x, -DT2, oxd2, OP.mult, OP.add),
                stt(c1y, DT2, oyd2, OP.mult, OP.add),
                stt(dtq[2], DT2, OZ, OP.mult, OP.add),
            ]

            # generic stage: given trig phase done for atI, compute
            # dynamics kI, accumulate, produce next stage state.
            ACC_p = [None, None, None]
            ACC_v = [None, None, None]
            ACC_a = [None, None, None]
            ACC_c = [None, None]  # weighted cross-term sums

            def ln_exp_phase(veI, atI_trig_cp, vrz_shift):
                """v_rel, drag root, secant for one stage."""
                vrI = [
                    tt(OP.subtract, veI[0], wdx_c, eng=GE),
                    tt(OP.subtract, veI[1], wdy_c, eng=GE),
                    tt(OP.subtract, veI[2], vrz_shift, eng=GE),
                ]
                s0 = act(vrI[0], AF.Square)
                s1 = act(vrI[1], AF.Square)
                s2_ = act(vrI[2], AF.Square)
                sv_ = tt(OP.add, s0, s1)
                tt(OP.add, sv_, s2_, out=sv_)
                ar.free(s0, s1, s2_)
                dro = act(sv_, AF.Sqrt, scale=0.0025)
                ar.free(sv_)
                sec = new()
                V.reciprocal(pl(sec), pl(atI_trig_cp))
                return vrI, dro, sec

            def accumulate(planes, weight, slot_list, base=None, eng=V):
                """ACC update: ACC = base + weight*planes (init) or
                ACC += weight*planes."""
                for i_, p_ in enumerate(planes):
                    if slot_list[i_] is None:
                        # init: ACC = weight*p + base_i
                        slot_list[i_] = stt(
                            p_, weight, base[i_], OP.mult, OP.add, eng=eng
                        )
                    else:
                        if weight == 1.0:
                            tt(
                                OP.add,
                                slot_list[i_],
                                p_,
                                out=slot_list[i_],
                                eng=eng,
                            )
                        else:
                            stt(
                                p_,
                                weight,
                                slot_list[i_],
                                OP.mult,
                                OP.add,
                                out=slot_list[i_],
                                eng=eng,
                            )

            # ===== k2 =====
            vr2, dro2, sec2 = ln_exp_phase(ve2, cp2, wdz_s2)
            rd2, pd2, yd2 = att_dot(sr2, cr2, sp2, sec2, *om2)
            acc2 = thrust_acc(sr2, cr2, sp2, cp2, sy2, cy2, dro2, vr2)
            ar.free(sr2, cr2, sp2, cp2, sy2, cy2, sec2, dro2, *vr2)
            c2x, c2y = cross_xy(*om2)
            # stage-3 state
            at3 = [
                stt(rd2, DT2, AR, OP.mult, OP.add),
                stt(pd2, DT2, AP_, OP.mult, OP.add),
                stt(yd2, DT2, AY, OP.mult, OP.add),
            ]
            ve3 = [
                stt(acc2[0], DT2, VX, OP.mult, OP.add),
                stt(acc2[1], DT2, VY, OP.mult, OP.add),
                stt(acc2[2], DT2, VZ, OP.mult, OP.add),
            ]
            om3 = [
                stt(c2x, -DT2, oxd2, OP.mult, OP.add),
                stt(c2y, DT2, oyd2, OP.mult, OP.add),
                stt(dtq[2], DT2, OZ, OP.mult, OP.add),
            ]
            # om2/oxd2/oyd2 are dead before the accumulates
            ar.free(*om2, oxd2, oyd2)
            # ACC init with k1 + 2*k2
            accumulate(ve2, 2.0, ACC_p, base=[VX, VY, VZ])
            accumulate(acc2, 2.0, ACC_v, base=acc1)
            accumulate([rd2, pd2, yd2], 2.0, ACC_a, base=[rd1, pd1, yd1])
            accumulate([c2x, c2y], 2.0, ACC_c, base=[c1x, c1y])
            ar.free(*ve2, *acc1, *acc2, rd1, pd1, yd1, rd2, pd2, yd2)
            ar.free(c1x, c1y, c2x, c2y)

            # ================= Phase T4: trig =================
            sr3 = act(at3[0], AF.Sin)
            cr3 = act(at3[0], AF.Sin, bias=HPI)
            sp3 = act(at3[1], AF.Sin)
            cp3 = act(at3[1], AF.Sin, bias=HPI)
            sy3 = act(at3[2], AF.Sin)
            cy3 = act(at3[2], AF.Sin, bias=HPI)
            ar.free(*at3)

            # ===== k3 =====
            vr3, dro3, sec3 = ln_exp_phase(ve3, cp3, wdz_s2)
            rd3, pd3, yd3 = att_dot(sr3, cr3, sp3, sec3, *om3)
            acc3 = thrust_acc(sr3, cr3, sp3, cp3, sy3, cy3, dro3, vr3)
            ar.free(sr3, cr3, sp3, cp3, sy3, cy3, sec3, dro3, *vr3)
            c3x, c3y = cross_xy(*om3)
            oxd4 = stt(dtq[0], DT, OX, OP.mult, OP.add)
            oyd4 = stt(dtq[1], DT, OY, OP.mult, OP.add)
            # stage-4 state (full dt)
            at4 = [
                stt(rd3, DT, AR, OP.mult, OP.add),
                stt(pd3, DT, AP_, OP.mult, OP.add),
                stt(yd3, DT, AY, OP.mult, OP.add),
            ]
            ve4 = [
                stt(acc3[0], DT, VX, OP.mult, OP.add),
                stt(acc3[1], DT, VY, OP.mult, OP.add),
                stt(acc3[2], DT, VZ, OP.mult, OP.add),
            ]
            om4 = [
                stt(c3x, -DT, oxd4, OP.mult, OP.add),
                stt(c3y, DT, oyd4, OP.mult, OP.add),
                stt(dtq[2], DT, OZ, OP.mult, OP.add),
            ]
            accumulate(ve3, 2.0, ACC_p)
            accumulate(acc3, 2.0, ACC_v)
            accumulate([rd3, pd3, yd3], 2.0, ACC_a)
            accumulate([c3x, c3y], 2.0, ACC_c)
            ar.free(*ve3, *om3, *acc3, rd3, pd3, yd3, c3x, c3y)

            # ================= Phase T6: trig =================
            sr4 = act(at4[0], AF.Sin)
            cr4 = act(at4[0], AF.Sin, bias=HPI)
            sp4 = act(at4[1], AF.Sin)
            cp4 = act(at4[1], AF.Sin, bias=HPI)
            sy4 = act(at4[2], AF.Sin)
            cy4 = act(at4[2], AF.Sin, bias=HPI)
            ar.free(*at4)

            # ===== k4 =====
            vr4, dro4, sec4 = ln_exp_phase(ve4, cp4, wdz_s4)
            rd4, pd4, yd4 = att_dot(sr4, cr4, sp4, sec4, *om4)
            acc4 = thrust_acc(sr4, cr4, sp4, cp4, sy4, cy4, dro4, vr4)
            ar.free(sr4, cr4, sp4, cp4, sy4, cy4, sec4, dro4, *vr4)
            c4x, c4y = cross_xy(*om4)
            accumulate(ve4, 1.0, ACC_p, eng=GE)
            accumulate(acc4, 1.0, ACC_v)
            accumulate([rd4, pd4, yd4], 1.0, ACC_a, eng=GE)
            accumulate([c4x, c4y], 1.0, ACC_c)
            ar.free(*ve4, *om4, *acc4, rd4, pd4, yd4, c4x, c4y)
            ar.free(wdz_s2, wdz_s4, wdx_c, wdy_c, T_)

            # ================= final combine =================
            def out_pl(c):
                return OUT[:, c * W : (c + 1) * W]

            # pos: z needs -3*dt*G correction folded into ACC
            V.tensor_scalar(
                pl(ACC_p[2]), pl(ACC_p[2]), -3.0 * DT * G, None, OP.add
            )
            # vel: z needs -6*G
            V.tensor_scalar(pl(ACC_v[2]), pl(ACC_v[2]), -6.0 * G, None, OP.add)
            # split the tail across DVE and Pool so the final combine
            # doesn't serialize on one engine before the out-DMA
            for j, (accs, base_c, eng_) in enumerate(
                [
                    (ACC_p, [PX, PY, PZ], V),
                    (ACC_v, [VX, VY, VZ], V),
                    (ACC_a, [AR, AP_, AY], V),
                ]
            ):
                for i_ in range(3):
                    eng_.scalar_tensor_tensor(
                        out_pl(j * 3 + i_),
                        pl(accs[i_]),
                        DT6,
                        IN[base_c[i_]],
                        OP.mult,
                        OP.add,
                    )
            # out_om = (om + dt*dtq) -/+ dt6 * weighted cross sums
            V.scalar_tensor_tensor(
                out_pl(9), pl(ACC_c[0]), -DT6, pl(oxd4), OP.mult, OP.add
            )
            V.scalar_tensor_tensor(
                out_pl(10), pl(ACC_c[1]), DT6, pl(oyd4), OP.mult, OP.add
            )
            V.scalar_tensor_tensor(
                out_pl(11), pl(dtq[2]), DT, IN[OZ], OP.mult, OP.add
            )
            ar.free(*ACC_p, *ACC_v, *ACC_a, *ACC_c, *dtq, oxd4, oyd4)

            # ---- store ----
            for c in range(N_OUT):
                nc.sync.dma_start(
                    out=yout[c, :, t * W : (t + 1) * W],
                    in_=OUT[:, c * W : (c + 1) * W],
                )

    nc.compile()
    return nc


def _prep_gains(inputs):
    f = np.float32
    kp_pos = np.abs(np.asarray(inputs["kp_pos"], f))
    ki_pos = np.abs(np.asarray(inputs["ki_pos"], f))
    kp_vel = np.abs(np.asarray(inputs["kp_vel"], f))
    ki_vel = np.abs(np.asarray(inputs["ki_vel"], f))
    kd_vel = np.abs(np.asarray(inputs["kd_vel"], f))
    kp_att = np.abs(np.asarray(inputs["kp_att"], f))
    kp_rate = np.abs(np.asarray(inputs["kp_rate"], f))
    ki_rate = np.abs(np.asarray(inputs["ki_rate"], f))
    kd_rate = np.abs(np.asarray(inputs["kd_rate"], f))
    inv_dt = np.float32(1.0) / np.float32(DT)
    return {
        "kp": [float(x) for x in kp_pos],
        "kip": [float(x) for x in ki_pos],
        "c1v": [float(np.float32(kp_vel[j]) + np.float32(kd_vel[j]) * inv_dt) for j in range(3)],
        "c2v": [float(x) for x in ki_vel],
        "c3v": [float(np.float32(kd_vel[j]) * inv_dt) for j in range(3)],
        "katt": [float(x) for x in kp_att],
        "c1r": [float(np.float32(kp_rate[j]) + np.float32(kd_rate[j]) * inv_dt) for j in range(3)],
        "c2r": [float(x) for x in ki_rate],
        "c3r": [float(np.float32(kd_rate[j]) * inv_dt) for j in range(3)],
    }


def pack_inputs(inputs, F):
    """Pack full inputs into per-core [34, 128, F] SoA planes."""
    f = np.float32
    B = B_TOTAL
    R = P * F
    tot = N_CORES * R
    state = np.asarray(inputs["state"], f)
    ch = np.empty((N_IN, tot), f)
    ch[0:12, :B] = state.T
    ch[12:15, :B] = np.asarray(inputs["target_pos"], f).T
    ch[15, :B] = np.asarray(inputs["target_yaw"], f)
    ch[16:19, :B] = np.asarray(inputs["wind"], f).T
    ch[19:22, :B] = np.asarray(inputs["prev_vel_err"], f).T
    ch[22:25, :B] = np.asarray(inputs["prev_rate_err"], f).T
    ch[25:28, :B] = np.asarray(inputs["integral_pos"], f).T
    ch[28:31, :B] = np.asarray(inputs["integral_vel"], f).T
    ch[31:34, :B] = np.asarray(inputs["integral_rate"], f).T
    pad = tot - B
    if pad:
        ch[:, B:] = ch[:, :pad]
    return ch.reshape(N_IN, N_CORES, P, F).transpose(1, 0, 2, 3).copy()


def unpack_outputs(results, F):
    """results: list of per-core dicts with 'yout' [12, 128, F]."""
    y = np.stack([np.asarray(r["yout"]) for r in results])  # [8,12,P,F]
    y = y.transpose(1, 0, 2, 3).reshape(N_OUT, N_CORES * P * F)
    return np.ascontiguousarray(y[:, :B_TOTAL].T)


def kernel(**inputs):
    F = 1956
    W = 652
    gains = _prep_gains(inputs)
    X = pack_inputs(inputs, F)
    nc = build_nc(F, W, gains)
    in_maps = [{"xin": X[c]} for c in range(N_CORES)]
    res = run_bass_kernel_spmd(nc, in_maps, list(range(N_CORES)))
    return unpack_outputs(res.results, F)


# revision 67
# speedup vs baseline: 1.1432x; 1.0124x over previous
"""Trainium2 Bass kernel for the DifferentiableDroneController problem.

Strategy:
  - Pure data parallelism across 8 NeuronCores (batch split).
  - Host-side SoA repack: every per-row channel becomes a contiguous
    [128, F] plane on the device, so all device ops are dense
    elementwise plane ops (no strided access anywhere).
  - All transcendentals on the Scalar engine using only TWO activation
    table sets:
      * trig_and_small: Sin (cos via bias=pi/2), Arctan (arcsin via
        arctan(x*rsqrt(1-x^2)))
      * natural_log_exp_and_others: 1/x = exp(-ln x),
        rsqrt(x) = exp(-0.5 ln x), sqrt(x) = exp(0.5 ln x)
  - Vector engine uses fused ops: scalar_tensor_tensor (a*c + b in one
    instruction) and dual-op tensor_scalar (clip in one instruction).
  - Algebraic simplifications: I_VEC cancels out of omega_dot entirely,
    gravity is folded into constants, the acc-norm clamp is
    min(1, 10*rsqrt(s)).
"""

import math
from contextlib import ExitStack

import numpy as np

import concourse.bacc as bacc
import concourse.bass as bass
import concourse.mybir as mybir
from concourse import tile
from concourse.bass_utils import run_bass_kernel_spmd

AF = mybir.ActivationFunctionType
OP = mybir.AluOpType
F32 = mybir.dt.float32

P = 128
N_CORES = 8
B_TOTAL = 2_000_000
DT = 0.01
DT2 = 0.005
DT6 = DT / 6.0
G = 9.81
PI = math.pi
HPI = math.pi / 2.0
LN10 = math.log(10.0)
LN005 = math.log(0.05)

# channel indices in the packed input [34, P, F]
PX, PY, PZ = 0, 1, 2
VX, VY, VZ = 3, 4, 5
AR, AP_, AY = 6, 7, 8
OX, OY, OZ = 9, 10, 11
TPX, TPY, TPZ = 12, 13, 14
TYAW = 15
WDX, WDY, WDZ = 16, 17, 18
PVX, PVY, PVZ = 19, 20, 21
PRX, PRY, PRZ = 22, 23, 24
IPX, IPY, IPZ = 25, 26, 27
IVX, IVY, IVZ = 28, 29, 30
IRX, IRY, IRZ = 31, 32, 33

N_IN = 34
N_OUT = 12


class Arena:
    """Manual plane allocator over a single [P, NA*W] SBUF tile."""

    def __init__(self, ap, W, n_slots, n_reserved):
        self.ap = ap
        self.W = W
        self.free_list = list(range(n_reserved, n_slots))
        self.peak = n_reserved
        self.n_slots = n_slots
        self.live = n_reserved

    def plane(self, slot):
        W = self.W
        return self.ap[:, slot * W : (slot + 1) * W]

    def alloc(self):
        assert self.free_list, "arena exhausted"
        s = self.free_list.pop(0)
        self.live += 1
        self.peak = max(self.peak, self.n_slots - len(self.free_list))
        return s

    def free(self, *slots):
        for s in slots:
            assert s not in self.free_list
            self.free_list.append(s)
            self.live -= 1


def build_nc(F, W, gains, use_gpsimd=True):
    """Build the Bass program. gains: dict with kp(3), kip(3), c1v(3),
    c2v(3), c3v(3), katt(3), c1r(3), c2r(3), c3r(3) as python floats."""
    assert F % W == 0
    n_tiles = F // W

    kp = gains["kp"]
    kip = gains["kip"]
    c1v = gains["c1v"]
    c2v = gains["c2v"]
    c3v = gains["c3v"]
    katt = gains["katt"]
    c1r = gains["c1r"]
    c2r = gains["c2r"]
    c3r = gains["c3r"]

    NA = 66  # arena slots (measured peak liveness is 63)

    nc = bacc.Bacc()

    # The act-table-load inserter picks the FIRST table set containing a
    # function. By default that maps Ln -> natural_log (no exp) and
    # Exp -> exp_and_others (no ln), so every ln/exp pair costs two
    # ~1.7us table loads. Remove those functions from the suboptimal
    # sets in the cached table dict so both resolve to
    # natural_log_exp_and_others, and arctan to trig_and_small (shared
    # with sin). Indices of the remaining sets are unchanged, so the
    # emitted act_func_set_ids stay valid for walrus.
    from concourse.hw_specs import get_activation_tables

    tabs = get_activation_tables(nc.m.arch)
    tabs["sigmoid_and_others"].discard(AF.Arctan)

    # register const APs for the activation biases we use
    for cval in (HPI, G, DT2 * G, DT * G):
        cten = nc.alloc_sbuf_tensor(f"constu-f32-{cval}", [P, 1], F32)
        nc.gpsimd.memset(cten.ap(), cval)
        nc.const_aps.aps[(F32, cval)] = cten.ap()
    nc.all_engine_barrier()

    xin = nc.declare_dram_parameter("xin", [N_IN, P, F], F32, isOutput=False)
    yout = nc.declare_dram_parameter("yout", [N_OUT, P, F], F32, isOutput=True)

    with tile.TileContext(nc) as tc, ExitStack() as ctx:
        arena_pool = ctx.enter_context(tc.tile_pool(name="arena", bufs=1))
        out_pool = ctx.enter_context(tc.tile_pool(name="out", bufs=1))
        psum_pool = ctx.enter_context(
            tc.tile_pool(name="pp", bufs=1, space="PSUM")
        )
        arena_tile = arena_pool.tile([P, NA * W], F32)

        V = nc.vector
        A = nc.scalar
        Gp = nc.gpsimd

        for t in range(n_tiles):
            ar = Arena(arena_tile[:], W, NA, N_IN)
            IN = [ar.plane(c) for c in range(N_IN)]

            # ---- load input planes (one DMA per channel keeps the
            # per-consumer sync-wait count within ISA limits); ordered
            # by first use so compute starts as early as possible ----
            dma_order = (
                [TYAW, AR, AP_, AY]
                + [TPX, TPY, TPZ, PX, PY, PZ]
                + [IPX, IPY, IPZ, VX, VY, VZ]
                + [IVX, IVY, IVZ, PVX, PVY, PVZ]
                + [OX, OY, OZ, IRX, IRY, IRZ]
                + [PRX, PRY, PRZ, WDX, WDY, WDZ]
            )
            for c in dma_order:
                nc.sync.dma_start(
                    out=arena_tile[:, c * W : (c + 1) * W],
                    in_=xin[c, :, t * W : (t + 1) * W],
                )

            OUT = out_pool.tile([P, N_OUT * W], F32)

            def pl(s):
                return ar.plane(s)

            def new():
                return ar.alloc()

            # engine helpers
            def tt(op, a, b, out=None, eng=V):
                o = out if out is not None else new()
                eng.tensor_tensor(pl(o), pl(a), pl(b), op)
                return o

            def ts(a, s1, op0, s2=None, op1=None, out=None, eng=V):
                o = out if out is not None else new()
                if s2 is None:
                    eng.tensor_scalar(pl(o), pl(a), s1, None, op0)
                else:
                    eng.tensor_scalar(pl(o), pl(a), s1, s2, op0, op1)
                return o

            def stt(a, s, b, op0, op1, out=None, eng=V):
                o = out if out is not None else new()
                eng.scalar_tensor_tensor(pl(o), pl(a), s, pl(b), op0, op1)
                return o

            def act(a, func, bias=0.0, scale=1.0, out=None):
                o = out if out is not None else new()
                A.activation(pl(o), pl(a), func, bias=bias, scale=scale)
                return o

            def clip_ip(a, lo, hi):
                V.tensor_scalar(pl(a), pl(a), lo, hi, OP.max, OP.min)
                return a

            # ================= Phase T0: trig =================
            # target_yaw spans (-pi, pi): sin(x+pi/2) would leave the
            # HW sin domain [-pi, pi], so cos via 1 - 2*sin^2(y/2).
            sty = act(TYAW, AF.Sin)
            syh = act(TYAW, AF.Sin, scale=0.5)
            sy2q = act(syh, AF.Square)
            ar.free(syh)
            cty = ts(sy2q, -2.0, OP.mult, 1.0, OP.add)
            ar.free(sy2q)
            sr1 = act(AR, AF.Sin)
            cr1 = act(AR, AF.Sin, bias=HPI)
            sp1 = act(AP_, AF.Sin)
            cp1 = act(AP_, AF.Sin, bias=HPI)
            sy1 = act(AY, AF.Sin)
            cy1 = act(AY, AF.Sin, bias=HPI)

            # ================= controller part 1 (vector) =================
            dac = []
            for j, (pj, tpj, ipj, vj, ivj, pvj) in enumerate(
                [
                    (PX, TPX, IPX, VX, IVX, PVX),
                    (PY, TPY, IPY, VY, IVY, PVY),
                    (PZ, TPZ, IPZ, VZ, IVZ, PVZ),
                ]
            ):
                pe = tt(OP.subtract, tpj, pj)
                ip2 = stt(pe, DT, ipj, OP.mult, OP.add)
                clip_ip(ip2, -2.0, 2.0)
                q = act(pe, AF.Copy, scale=kp[j])
                ar.free(pe)
                dv = stt(ip2, kip[j], q, OP.mult, OP.add)
                ar.free(ip2, q)
                clip_ip(dv, -10.0, 10.0)
                ve = tt(OP.subtract, dv, vj)
                ar.free(dv)
                iv2 = stt(ve, DT, ivj, OP.mult, OP.add)
                clip_ip(iv2, -2.0, 2.0)
                q2 = act(ve, AF.Copy, scale=c1v[j])
                q3 = stt(iv2, c2v[j], q2, OP.mult, OP.add)
                ar.free(iv2, q2, ve)
                dacj = stt(pvj, -c3v[j], q3, OP.mult, OP.add)
                ar.free(q3)
                # controller-only input channels are dead now
                ar.free(tpj, ipj, ivj, pvj)
                dac.append(dacj)

            # stash pos in PSUM so its input slots stop being read after
            # the controller; next tile's pos DMAs then overlap this tile
            ppos = psum_pool.tile([P, 3 * W], F32)
            for j_ in range(3):
                A.activation(
                    ppos[:, j_ * W : (j_ + 1) * W], IN[PX + j_], AF.Copy
                )

            # v_rel for k1 (vector, independent)
            vr1 = [
                tt(OP.subtract, VX, WDX),
                tt(OP.subtract, VY, WDY),
                tt(OP.subtract, VZ, WDZ),
            ]
            # shifted wind-z for later stages (fold gravity); scratch
            # copies of wind-x/y release the input slots early so the
            # next tile's wind DMAs overlap this tile's dynamics
            wdz_s2 = act(WDZ, AF.Identity, bias=DT2 * G)
            wdz_s4 = act(WDZ, AF.Identity, bias=DT * G)
            wdx_c = act(WDX, AF.Copy)
            wdy_c = act(WDY, AF.Copy)

            # ================= Phase L1: sqrt + reciprocal =========
            # acceleration-norm clamp factor = min(1, 10*rsqrt(s))
            sq0 = act(dac[0], AF.Square)
            sq1 = act(dac[1], AF.Square)
            sq2 = act(dac[2], AF.Square)
            sacc = tt(OP.add, sq0, sq1)
            tt(OP.add, sacc, sq2, out=sacc)
            ar.free(sq0, sq1, sq2)
            # sqrt(0.01*s) = 0.1*sqrt(s); recip -> 10/sqrt(s)
            racc = act(sacc, AF.Sqrt, scale=0.01)
            f10 = new()
            V.reciprocal(pl(f10), pl(racc))
            ar.free(sacc, racc)
            V.tensor_scalar(pl(f10), pl(f10), 1.0, None, OP.min)
            for j in range(3):
                tt(OP.mult, dac[j], f10, out=dac[j])
            ar.free(f10)
            # thrust vector: z gets +G (squares fold it via bias);
            # tn[:,2] is never used by the reference, so no explicit tvz.
            q0 = act(dac[0], AF.Square)
            q1 = act(dac[1], AF.Square)
            q2_ = act(dac[2], AF.Square, bias=G)
            s2 = tt(OP.add, q0, q1)
            tt(OP.add, s2, q2_, out=s2)
            ar.free(q0, q1, q2_)
            Tn = act(s2, AF.Sqrt)  # sqrt(s2) unclipped
            ar.free(s2)
            rsq2 = new()
            V.reciprocal(pl(rsq2), pl(Tn))
            T_ = ts(Tn, 0.1 * G, OP.max, 2.0 * G, OP.min)
            ar.free(Tn)
            tnx = tt(OP.mult, dac[0], rsq2)
            tny = tt(OP.mult, dac[1], rsq2)
            ar.free(dac[0], dac[1], dac[2], rsq2)
            # roll_arg
            ra = tt(OP.mult, tnx, sty)
            rb = tt(OP.mult, tny, cty)
            u = tt(OP.subtract, ra, rb)
            ar.free(ra, rb)
            # arcsin(u) = 2*arctan(u / (1 + sqrt(1-u^2))) keeps the
            # arctan argument inside the HW domain [-pi/2, pi/2].
            clip_ip(u, -0.999, 0.999)
            u2 = act(u, AF.Square)
            wu = act(u2, AF.Sqrt, bias=1.0, scale=-1.0)  # cos(des_roll)
            ar.free(u2)
            mru = new()
            V.reciprocal(pl(mru), pl(wu))  # 1/cos(des_roll)
            dnu = act(wu, AF.Identity, bias=1.0)
            ar.free(wu)
            rdu = new()
            V.reciprocal(pl(rdu), pl(dnu))
            ar.free(dnu)
            uarg = tt(OP.mult, u, rdu)
            ar.free(u, rdu)
            # pitch_arg
            pa = tt(OP.mult, tnx, cty)
            pb = tt(OP.mult, tny, sty)
            ar.free(tnx, tny, sty, cty)
            pc = tt(OP.add, pa, pb)
            v_ = tt(OP.mult, pc, mru)
            ar.free(pa, pb, pc, mru)
            clip_ip(v_, -0.999, 0.999)
            v2 = act(v_, AF.Square)
            wv = act(v2, AF.Sqrt, bias=1.0, scale=-1.0)
            ar.free(v2)
            dnv = act(wv, AF.Identity, bias=1.0)
            ar.free(wv)
            rdv = new()
            V.reciprocal(pl(rdv), pl(dnv))
            ar.free(dnv)
            varg = tt(OP.mult, v_, rdv)
            ar.free(v_, rdv)
            # k1 drag root and secant
            sv0 = act(vr1[0], AF.Square)
            sv1_ = act(vr1[1], AF.Square)
            sv2 = act(vr1[2], AF.Square)
            sv = tt(OP.add, sv0, sv1_)
            tt(OP.add, sv, sv2, out=sv)
            ar.free(sv0, sv1_, sv2)
            # 0.05*|v| = sqrt(0.0025*|v|^2)
            dro1 = act(sv, AF.Sqrt, scale=0.0025)
            ar.free(sv)
            sec1 = new()
            V.reciprocal(pl(sec1), pl(cp1))

            # ---------- dynamics helper (after trig + sec/dro ready) ----
            GE = Gp if use_gpsimd else V

            def att_dot(sr, cr, sp, sec, omx, omy, omz):
                m1 = tt(OP.mult, sr, omy, eng=GE)
                m2 = tt(OP.mult, cr, omz, eng=GE)
                m3 = tt(OP.add, m1, m2, eng=GE)
                ar.free(m1, m2)
                yd = tt(OP.mult, m3, sec, eng=GE)
                ar.free(m3)
                # roll_dot = wx + (sp*sec)*m3 = wx + sp*yd
                rda = tt(OP.mult, sp, yd, eng=GE)
                rd = tt(OP.add, rda, omx)
                ar.free(rda)
                pda = tt(OP.mult, cr, omy, eng=GE)
                pdb = tt(OP.mult, sr, omz, eng=GE)
                pd = tt(OP.subtract, pda, pdb, eng=GE)
                ar.free(pda, pdb)
                return rd, pd, yd

            def thrust_acc(sr, cr, sp, cp, sy, cy, dro, vr):
                # fold T early: m=T*cr shared by t1 (T*sp*cr) and tgz
                m = tt(OP.mult, T_, cr, eng=GE)
                t1 = tt(OP.mult, sp, m, eng=GE)
                r = tt(OP.mult, T_, sr, eng=GE)
                a_ = tt(OP.mult, cy, t1, eng=GE)
                b_ = tt(OP.mult, sy, r, eng=GE)
                tgx = tt(OP.add, a_, b_, eng=GE)
                ar.free(a_, b_)
                c_ = tt(OP.mult, sy, t1, eng=GE)
                d_ = tt(OP.mult, cy, r, eng=GE)
                ar.free(t1, r)
                tgy = tt(OP.subtract, c_, d_, eng=GE)
                ar.free(c_, d_)
                tgz = tt(OP.mult, cp, m, eng=GE)
                ar.free(m)
                accs = []
                for tg, vrj in zip((tgx, tgy, tgz), vr):
                    dr = tt(OP.mult, dro, vrj, eng=GE)
                    acc = tt(OP.subtract, tg, dr, eng=GE)
                    ar.free(tg, dr)
                    accs.append(acc)
                return accs

            def cross_xy(omx, omy, omz):
                # omega_dot_x = dtqx - cx, omega_dot_y = dtqy + cy;
                # the dtq part is hoisted into oxd2/oxd4 below, so only
                # the cross terms are computed per stage.
                cx = tt(OP.mult, omy, omz, eng=GE)
                cy_ = tt(OP.mult, omx, omz, eng=GE)
                return cx, cy_

            # k1 attitude dynamics (needs only state + sec1)
            rd1, pd1, yd1 = att_dot(sr1, cr1, sp1, sec1, OX, OY, OZ)
            # stage-2 attitude
            at2 = [
                stt(rd1, DT2, AR, OP.mult, OP.add),
                stt(pd1, DT2, AP_, OP.mult, OP.add),
                stt(yd1, DT2, AY, OP.mult, OP.add),
            ]

            # ================= Phase T2: trig =================
            droll = act(uarg, AF.Arctan)
            dpitch = act(varg, AF.Arctan)
            ar.free(uarg, varg)
            sr2 = act(at2[0], AF.Sin)
            cr2 = act(at2[0], AF.Sin, bias=HPI)
            sp2 = act(at2[1], AF.Sin)
            cp2 = act(at2[1], AF.Sin, bias=HPI)
            sy2 = act(at2[2], AF.Sin)
            cy2 = act(at2[2], AF.Sin, bias=HPI)
            ar.free(*at2)

            # ---- controller part 2 (vector) ----
            # droll/dpitch hold atan(tan(theta/2)); clip at 0.523/2 and
            # fold the *2 into the attitude-error subtraction.
            half_clip = float(np.float32(0.523) / np.float32(2.0))
            clip_ip(droll, -half_clip, half_clip)
            clip_ip(dpitch, -half_clip, half_clip)
            aer = stt(droll, 2.0, AR, OP.mult, OP.subtract)
            aep = stt(dpitch, 2.0, AP_, OP.mult, OP.subtract)
            ar.free(droll, dpitch)
            x_ = tt(OP.subtract, TYAW, AY)
            g1 = ts(x_, PI, OP.is_gt, 2.0 * PI, OP.mult)
            g2 = ts(x_, -PI, OP.is_lt, 2.0 * PI, OP.mult)
            x1 = stt(g1, -1.0, x_, OP.mult, OP.add)
            ar.free(g1, x_)
            aey = tt(OP.add, x1, g2)
            ar.free(x1, g2, TYAW)
            dtq = []
            for j, (aej, oj, irj, prj) in enumerate(
                [(aer, OX, IRX, PRX), (aep, OY, IRY, PRY), (aey, OZ, IRZ, PRZ)]
            ):
                re = stt(aej, katt[j], oj, OP.mult, OP.subtract)
                ar.free(aej)
                ir2 = stt(re, DT, irj, OP.mult, OP.add)
                clip_ip(ir2, -1.0, 1.0)
                q = act(re, AF.Copy, scale=c1r[j])
                q2 = stt(ir2, c2r[j], q, OP.mult, OP.add)
                ar.free(ir2, q, re)
                dtqj = stt(prj, -c3r[j], q2, OP.mult, OP.add)
                ar.free(q2, irj, prj)
                dtq.append(dtqj)

            # ---- finish k1 (vector) ----
            # hoisted omega + dt*dtq terms (shared by stages and final)
            oxd2 = stt(dtq[0], DT2, OX, OP.mult, OP.add)
            oyd2 = stt(dtq[1], DT2, OY, OP.mult, OP.add)
            acc1 = thrust_acc(sr1, cr1, sp1, cp1, sy1, cy1, dro1, vr1)
            ar.free(sr1, cr1, sp1, cp1, sy1, cy1, sec1, dro1, *vr1)
            c1x, c1y = cross_xy(OX, OY, OZ)
            # stage-2 velocity / omega
            ve2 = [
                stt(acc1[0], DT2, VX, OP.mult, OP.add),
                stt(acc1[1], DT2, VY, OP.mult, OP.add),
                stt(acc1[2], DT2, VZ, OP.mult, OP.add),
            ]
            om2 = [
                stt(c1x, -DT2, oxd2, OP.mult, OP.add),
                stt(c1y, DT2, oyd2, OP.mult, OP.add),
                stt(dtq[2], DT2, OZ, OP.mult, OP.add),
            ]

            # generic stage: given trig phase done for atI, compute
            # dynamics kI, accumulate, produce next stage state.
            ACC_p = [None, None, None]
            ACC_v = [None, None, None]
            ACC_a = [None, None, None]
            ACC_c = [None, None]  # weighted cross-term sums

            def ln_exp_phase(veI, atI_trig_cp, vrz_shift):
                """v_rel, drag root, secant for one stage."""
                vrI = [
                    tt(OP.subtract, veI[0], wdx_c, eng=GE),
                    tt(OP.subtract, veI[1], wdy_c, eng=GE),
                    tt(OP.subtract, veI[2], vrz_shift, eng=GE),
                ]
                s0 = act(vrI[0], AF.Square)
                s1 = act(vrI[1], AF.Square)
                s2_ = act(vrI[2], AF.Square)
                sv_ = tt(OP.add, s0, s1)
                tt(OP.add, sv_, s2_, out=sv_)
                ar.free(s0, s1, s2_)
                dro = act(sv_, AF.Sqrt, scale=0.0025)
                ar.free(sv_)
                sec = new()
                V.reciprocal(pl(sec), pl(atI_trig_cp))
                return vrI, dro, sec

            def accumulate(planes, weight, slot_list, base=None, eng=V):
                """ACC update: ACC = base + weight*planes (init) or
                ACC += weight*planes."""
                for i_, p_ in enumerate(planes):
                    if slot_list[i_] is None:
                        # init: ACC = weight*p + base_i
                        slot_list[i_] = stt(
                            p_, weight, base[i_], OP.mult, OP.add, eng=eng
                        )
                    else:
                        if weight == 1.0:
                            tt(
                                OP.add,
                                slot_list[i_],
                                p_,
                                out=slot_list[i_],
                                eng=eng,
                            )
                        else:
                            stt(
                                p_,
                                weight,
                                slot_list[i_],
                                OP.mult,
                                OP.add,
                                out=slot_list[i_],
                                eng=eng,
                            )

            # ===== k2 =====
            vr2, dro2, sec2 = ln_exp_phase(ve2, cp2, wdz_s2)
            rd2, pd2, yd2 = att_dot(sr2, cr2, sp2, sec2, *om2)
            acc2 = thrust_acc(sr2, cr2, sp2, cp2, sy2, cy2, dro2, vr2)
            ar.free(sr2, cr2, sp2, cp2, sy2, cy2, sec2, dro2, *vr2)
            c2x, c2y = cross_xy(*om2)
            # stage-3 state
            at3 = [
                stt(rd2, DT2, AR, OP.mult, OP.add),
                stt(pd2, DT2, AP_, OP.mult, OP.add),
                stt(yd2, DT2, AY, OP.mult, OP.add),
            ]
            ve3 = [
                stt(acc2[0], DT2, VX, OP.mult, OP.add),
                stt(acc2[1], DT2, VY, OP.mult, OP.add),
                stt(acc2[2], DT2, VZ, OP.mult, OP.add),
            ]
            om3 = [
                stt(c2x, -DT2, oxd2, OP.mult, OP.add),
                stt(c2y, DT2, oyd2, OP.mult, OP.add),
                stt(dtq[2], DT2, OZ, OP.mult, OP.add),
            ]
            # om2/oxd2/oyd2 are dead before the accumulates
            ar.free(*om2, oxd2, oyd2)
            # ACC init with k1 + 2*k2
            accumulate(ve2, 2.0, ACC_p, base=[VX, VY, VZ])
            accumulate(acc2, 2.0, ACC_v, base=acc1)
            accumulate([rd2, pd2, yd2], 2.0, ACC_a, base=[rd1, pd1, yd1])
            accumulate([c2x, c2y], 2.0, ACC_c, base=[c1x, c1y])
            ar.free(*ve2, *acc1, *acc2, rd1, pd1, yd1, rd2, pd2, yd2)
            ar.free(c1x, c1y, c2x, c2y)

            # ================= Phase T4: trig =================
            sr3 = act(at3[0], AF.Sin)
            cr3 = act(at3[0], AF.Sin, bias=HPI)
            sp3 = act(at3[1], AF.Sin)
            cp3 = act(at3[1], AF.Sin, bias=HPI)
            sy3 = act(at3[2], AF.Sin)
            cy3 = act(at3[2], AF.Sin, bias=HPI)
            ar.free(*at3)

            # ===== k3 =====
            vr3, dro3, sec3 = ln_exp_phase(ve3, cp3, wdz_s2)
            rd3, pd3, yd3 = att_dot(sr3, cr3, sp3, sec3, *om3)
            acc3 = thrust_acc(sr3, cr3, sp3, cp3, sy3, cy3, dro3, vr3)
            ar.free(sr3, cr3, sp3, cp3, sy3, cy3, sec3, dro3, *vr3)
            c3x, c3y = cross_xy(*om3)
            oxd4 = stt(dtq[0], DT, OX, OP.mult, OP.add)
            oyd4 = stt(dtq[1], DT, OY, OP.mult, OP.add)
            # stage-4 state (full dt)
            at4 = [
                stt(rd3, DT, AR, OP.mult, OP.add),
                stt(pd3, DT, AP_, OP.mult, OP.add),
                stt(yd3, DT, AY, OP.mult, OP.add),
            ]
            ve4 = [
                stt(acc3[0], DT, VX, OP.mult, OP.add),
                stt(acc3[1], DT, VY, OP.mult, OP.add),
                stt(acc3[2], DT, VZ, OP.mult, OP.add),
            ]
            om4 = [
                stt(c3x, -DT, oxd4, OP.mult, OP.add),
                stt(c3y, DT, oyd4, OP.mult, OP.add),
                stt(dtq[2], DT, OZ, OP.mult, OP.add),
            ]
            accumulate(ve3, 2.0, ACC_p)
            accumulate(acc3, 2.0, ACC_v)
            accumulate([rd3, pd3, yd3], 2.0, ACC_a)
            accumulate([c3x, c3y], 2.0, ACC_c)
            ar.free(*ve3, *om3, *acc3, rd3, pd3, yd3, c3x, c3y)

            # ================= Phase T6: trig =================
            sr4 = act(at4[0], AF.Sin)
            cr4 = act(at4[0], AF.Sin, bias=HPI)
            sp4 = act(at4[1], AF.Sin)
            cp4 = act(at4[1], AF.Sin, bias=HPI)
            sy4 = act(at4[2], AF.Sin)
            cy4 = act(at4[2], AF.Sin, bias=HPI)
            ar.free(*at4)

            # ===== k4 =====
            vr4, dro4, sec4 = ln_exp_phase(ve4, cp4, wdz_s4)
            rd4, pd4, yd4 = att_dot(sr4, cr4, sp4, sec4, *om4)
            acc4 = thrust_acc(sr4, cr4, sp4, cp4, sy4, cy4, dro4, vr4)
            ar.free(sr4, cr4, sp4, cp4, sy4, cy4, sec4, dro4, *vr4)
            c4x, c4y = cross_xy(*om4)
            accumulate(ve4, 1.0, ACC_p, eng=GE)
            accumulate(acc4, 1.0, ACC_v)
            accumulate([rd4, pd4, yd4], 1.0, ACC_a, eng=GE)
            accumulate([c4x, c4y], 1.0, ACC_c)
            ar.free(*ve4, *om4, *acc4, rd4, pd4, yd4, c4x, c4y)
            ar.free(wdz_s2, wdz_s4, wdx_c, wdy_c, T_)

            # ================= final combine =================
            def out_pl(c):
                return OUT[:, c * W : (c + 1) * W]

            # pos: z needs -3*dt*G correction folded into ACC
            V.tensor_scalar(
                pl(ACC_p[2]), pl(ACC_p[2]), -3.0 * DT * G, None, OP.add
            )
            # vel: z needs -6*G
            V.tensor_scalar(pl(ACC_v[2]), pl(ACC_v[2]), -6.0 * G, None, OP.add)
            # split the tail across DVE and Pool so the final combine
            # doesn't serialize on one engine before the out-DMA
            for i_ in range(3):
                V.scalar_tensor_tensor(
                    out_pl(i_),
                    pl(ACC_p[i_]),
                    DT6,
                    ppos[:, i_ * W : (i_ + 1) * W],
                    OP.mult,
                    OP.add,
                )
            for j, (accs, base_c) in enumerate(
                [(ACC_v, [VX, VY, VZ]), (ACC_a, [AR, AP_, AY])]
            ):
                for i_ in range(3):
                    V.scalar_tensor_tensor(
                        out_pl(3 + j * 3 + i_),
                        pl(accs[i_]),
                        DT6,
                        IN[base_c[i_]],
                        OP.mult,
                        OP.add,
                    )
            # out_om = (om + dt*dtq) -/+ dt6 * weighted cross sums
            V.scalar_tensor_tensor(
                out_pl(9), pl(ACC_c[0]), -DT6, pl(oxd4), OP.mult, OP.add
            )
            V.scalar_tensor_tensor(
                out_pl(10), pl(ACC_c[1]), DT6, pl(oyd4), OP.mult, OP.add
            )
            V.scalar_tensor_tensor(
                out_pl(11), pl(dtq[2]), DT, IN[OZ], OP.mult, OP.add
            )
            ar.free(*ACC_p, *ACC_v, *ACC_a, *ACC_c, *dtq, oxd4, oyd4)

            # ---- store ----
            for c in range(N_OUT):
                nc.sync.dma_start(
                    out=yout[c, :, t * W : (t + 1) * W],
                    in_=OUT[:, c * W : (c + 1) * W],
                )

    nc.compile()
    return nc


def _prep_gains(inputs):
    f = np.float32
    kp_pos = np.abs(np.asarray(inputs["kp_pos"], f))
    ki_pos = np.abs(np.asarray(inputs["ki_pos"], f))
    kp_vel = np.abs(np.asarray(inputs["kp_vel"], f))
    ki_vel = np.abs(np.asarray(inputs["ki_vel"], f))
    kd_vel = np.abs(np.asarray(inputs["kd_vel"], f))
    kp_att = np.abs(np.asarray(inputs["kp_att"], f))
    kp_rate = np.abs(np.asarray(inputs["kp_rate"], f))
    ki_rate = np.abs(np.asarray(inputs["ki_rate"], f))
    kd_rate = np.abs(np.asarray(inputs["kd_rate"], f))
    inv_dt = np.float32(1.0) / np.float32(DT)
    return {
        "kp": [float(x) for x in kp_pos],
        "kip": [float(x) for x in ki_pos],
        "c1v": [float(np.float32(kp_vel[j]) + np.float32(kd_vel[j]) * inv_dt) for j in range(3)],
        "c2v": [float(x) for x in ki_vel],
        "c3v": [float(np.float32(kd_vel[j]) * inv_dt) for j in range(3)],
        "katt": [float(x) for x in kp_att],
        "c1r": [float(np.float32(kp_rate[j]) + np.float32(kd_rate[j]) * inv_dt) for j in range(3)],
        "c2r": [float(x) for x in ki_rate],
        "c3r": [float(np.float32(kd_rate[j]) * inv_dt) for j in range(3)],
    }


def pack_inputs(inputs, F):
    """Pack full inputs into per-core [34, 128, F] SoA planes."""
    f = np.float32
    B = B_TOTAL
    R = P * F
    tot = N_CORES * R
    state = np.asarray(inputs["state"], f)
    ch = np.empty((N_IN, tot), f)
    ch[0:12, :B] = state.T
    ch[12:15, :B] = np.asarray(inputs["target_pos"], f).T
    ch[15, :B] = np.asarray(inputs["target_yaw"], f)
    ch[16:19, :B] = np.asarray(inputs["wind"], f).T
    ch[19:22, :B] = np.asarray(inputs["prev_vel_err"], f).T
    ch[22:25, :B] = np.asarray(inputs["prev_rate_err"], f).T
    ch[25:28, :B] = np.asarray(inputs["integral_pos"], f).T
    ch[28:31, :B] = np.asarray(inputs["integral_vel"], f).T
    ch[31:34, :B] = np.asarray(inputs["integral_rate"], f).T
    pad = tot - B
    if pad:
        ch[:, B:] = ch[:, :pad]
    return ch.reshape(N_IN, N_CORES, P, F).transpose(1, 0, 2, 3).copy()


def unpack_outputs(results, F):
    """results: list of per-core dicts with 'yout' [12, 128, F]."""
    y = np.stack([np.asarray(r["yout"]) for r in results])  # [8,12,P,F]
    y = y.transpose(1, 0, 2, 3).reshape(N_OUT, N_CORES * P * F)
    return np.ascontiguousarray(y[:, :B_TOTAL].T)


def kernel(**inputs):
    F = 1956
    W = 652
    gains = _prep_gains(inputs)
    X = pack_inputs(inputs, F)
    nc = build_nc(F, W, gains)
    in_maps = [{"xin": X[c]} for c in range(N_CORES)]
    res = run_bass_kernel_spmd(nc, in_maps, list(range(N_CORES)))
    return unpack_outputs(res.results, F)
